# revision 95
# baseline (speedup 1.0000x reference)
"""Trainium2 Bass kernel for a SegFormer-style transformer block.

Reference computation (per batch element b):
    x  = x + attention(LN1(x))          # 8 heads, d=64, no qkv bias
    x  = x + mixffn(LN2(x))             # fc1 -> dwconv3x3 -> gelu -> fc2

Sharding: pure data-parallel over batch B=8 across the 8 NeuronCores
(one batch element per core, weights replicated, no collectives).

Per-core layout strategy (v3):
  - LayerNorm stats run token-major; the transpose to feature-major goes
    through the DMA XBAR (dma_start_transpose, 16x128 tiles, two token
    tiles per transfer) instead of the PE.
  - Attention processes heads in PAIRS: head 2t lives on partitions 0:64,
    head 2t+1 on 64:128.  Score matmuls (K=64) for the two heads run
    CONCURRENTLY in the PE array via row tile_position (0,0)/(64,0);
    A@V matmuls (M=64) run concurrently via col tile_position (0,0)/(0,64).
    Softmax denominators come from an all-ones [128,64] stationary matmul
    (output = denominator broadcast across 64 partitions, same PSUM bank
    as A@V's pair so the approximate reciprocal runs at base partition 0).
  - The attention is software-pipelined: the exp evacuations pace the
    phase, so pair t's scores interleave with pair t-1's A@V/denominator
    matmuls (pair 0 interleaves with the V projection); E tiles are
    double-buffered across pairs.
  - proj contracts the full 128-dim head pair in one accumulation chain.
  - The depthwise 3x3 conv runs on the PE as 9 diagonal-matrix matmuls
    accumulating in PSUM; diag tiles are built on the DVE in bf16 (4x).
  - PSUM budget (8 banks): pbig 2x[P,1024] + pav 4x[P,512]; matmul
    chains pair both query chunks into one pbig tile so every ACT/DVE
    evacuation is a single 1024-wide instruction (ACT has ~352 cycles of
    fixed overhead per instruction).
  - All matmuls in bf16 (fp32 PSUM accumulation).

Self-contained: hardcodes all shapes; takes full inputs, returns full
output.
"""

import numpy as np
import ml_dtypes

import concourse.bass as bass
import concourse.tile as tile
from concourse import bacc, mybir
from concourse import bass_utils
from concourse.bass import ts, ds
from concourse.masks import make_identity

P = 128
NTOK = 1024
C = 512
HID = 2048
NH = 8
NP = NH // 2        # head pairs
D = 64
HH = 32
WW = 32
NT = NTOK // P      # 8 token tiles
CT = C // P         # 4 feature tiles
HCT = HID // P      # 16 hidden tiles
EPS = 1e-5
N_CORES = 8

f32 = mybir.dt.float32
bf16 = mybir.dt.bfloat16
fp8 = mybir.dt.float8e4
u8 = mybir.dt.uint8
# Schraudolph fast-exp constants: trunc(x*8/ln2 + 56.16) as uint8 bits IS
# fp8e4m3(~e^x); beta tuned for min max log-ratio error under truncation
SCH_ALPHA = 8.0 / 0.6931471805599453
SCH_BETA = 56.16
AF = mybir.ActivationFunctionType
OP = mybir.AluOpType
DR = mybir.MatmulPerfMode.DoubleRow
W8SCALE = 16.0  # fp8 weights are scaled up x16 (host) and back /16 at evac

# ---- depthwise conv v2: fp8 DoubleRow tap-pair matmuls ----
# ht tile layout (per hidden chunk): [P, 3 planes, CV_S] fp8.
#   plane1 = original zero-padded 34x34 grid at byte CV_G (written by the
#            fc1 evacuation, interior rows/cols 1..32 only; pads stay zero
#            from the one-time whole-tile memset).
#   plane0 = plane1 shifted by +2 elements (DMA copy), plane2 = shifted -34.
# A DR matmul reads two planes at the same in-plane offset x0 (middle-dim
# stride CV_S must be %16==0), so one matmul applies TWO taps:
#   via plane1 (orig):    tap delta = x0 - CV_G
#   via plane0 (shift+2): tap delta = x0 - CV_G + 2
#   via plane2 (shift-34): tap delta = x0 - CV_G - 34
# tap (ky,kx) -> delta = (ky-1)*34 + (kx-1) on the padded grid.
CV_S = 1232            # plane stride (fp8 bytes), 77*16
CV_G = 64              # grid origin within a plane (apron before it)
CV_NPAD = 34 * 34      # padded grid size (1156)
# (first_ap_plane, x0, tap_on_q0, tap_on_q1); taps as (ky,kx), None = zeros
CV_PAIRS = [
    (0, 29, (0, 2), (0, 0)),   # deltas -33, -35
    (1, 64, (1, 1), (0, 1)),   # planes (1,2): deltas 0, -34
    (0, 63, (1, 2), (1, 0)),   # deltas +1, -1
    (0, 97, (2, 2), (2, 0)),   # deltas +35, +33
    (0, 98, None, (2, 1)),     # deltas (+36 x zeros), +34
]


def _emit(tc, d, out_ap, gelu_mode="hw", dbg=None):
    def dump(key, ap):
        if dbg is not None and key in dbg:
            tc.nc.sync.dma_start(dbg[key], ap)

    nc = tc.nc

    # ---- whole-kernel pools ----
    pool_const = tc.alloc_tile_pool(name="const", bufs=1)
    pool_x = tc.alloc_tile_pool(name="x", bufs=1)
    pool_x1 = tc.alloc_tile_pool(name="x1", bufs=1)
    pool_stats = tc.alloc_tile_pool(name="stats", bufs=4)
    pool_out = tc.alloc_tile_pool(name="outp", bufs=1)
    # PSUM budget (8 banks): pbig 2x[P,1024] + pav (av,dn) x2 [P,512]
    pool_pbig = tc.alloc_tile_pool(name="pbig", bufs=2, space="PSUM")
    pool_pav = tc.alloc_tile_pool(name="pav", bufs=2, space="PSUM")

    identb = pool_const.tile([P, P], bf16, tag="identb", name="identb")
    make_identity(nc, identb[:])
    # fp8 all-ones stationary for the denominator matmuls
    ones8r = pool_const.tile([P, 2, D], fp8, tag="ones8r", name="ones8r")
    nc.vector.memset(ones8r[:], 1.0)
    zconst = pool_const.tile([P, 1], f32, tag="zconst", name="zconst")
    nc.vector.memset(zconst[:], 0.0)
    nc.const_aps.aps[(f32, 0.0)] = zconst[:]
    epsap = pool_const.tile([P, 1], f32, tag="epsap", name="epsap")
    nc.vector.memset(epsap[:], EPS)

    # all per-partition bias vectors arrive host-packed in one [P, 56] DMA
    # (small, ahead of the x tiles on the sync queue)
    bias_sb = pool_const.tile([P, 6 * CT + 3 * HCT], f32, tag="biases", name="biases")
    nc.sync.dma_start(bias_sb[:], d["biases"])
    g1pp = bias_sb[:, 0:4]
    b1pp = bias_sb[:, 4:8]
    g2pp = bias_sb[:, 8:12]
    b2pp = bias_sb[:, 12:16]
    pbpp = bias_sb[:, 16:20]
    f2bpp = bias_sb[:, 20:24]
    f1bpp = bias_sb[:, 24:40]
    dwbpp = bias_sb[:, 40:56]
    dwtpp = bias_sb[:, 56:72]
    # host-prebuilt fp8 diagonal conv weights: [P, hc, pair, plane, 128].
    # The tile lives in the const pool but its (bulky) DMA is emitted later,
    # once the latency-critical LN1 transfers have cleared the queues.
    dw8 = pool_const.tile([P, HCT, 5, 2, P], fp8, tag="dw8", name="dw8")
    # persistent triple-buffered conv input tiles; zero-filled once via DMA
    # (engine-free) so pads/aprons/tails read as zeros forever (the fc1 evac
    # only writes interiors, the plane copies rewrite full grids).
    pool_htp = tc.alloc_tile_pool(name="htp", bufs=1, side="right")
    ht_bufs = [
        pool_htp.tile([P, 3, CV_S], fp8, tag=f"htb{i}", name=f"htb{i}")
        for i in range(3)
    ]
    # pre-trigger the sqrt table set (LN rstd) while the input DMAs run
    scr1 = pool_const.tile([P, 1], f32, tag="scr1", name="scr1")
    nc.scalar.activation(scr1[0:1, :], epsap[0:1, :], AF.Sqrt)

    # x arrives host-repacked partition-major bf16 [P, NT, C]: two DMAs with
    # 8KB-contiguous partition lines instead of 1024 2KB descriptors.
    xall = pool_x.tile([P, NT, C], bf16, tag="xall", name="xall")
    nc.sync.dma_start(xall[:, 0:1, :], d["xb"][:, 0:1, :])
    nc.sync.dma_start(xall[:, 1:4, :], d["xb"][:, 1:4, :])
    nc.scalar.dma_start(xall[:, 4:8, :], d["xb"][:, 4:8, :])
    x_sb = [xall[:, i, :] for i in range(NT)]
    x1_sb = [pool_x1.tile([P, C], f32, tag=f"x1_{i}", name=f"x1_{i}") for i in range(NT)]
    # fc1/fc2 weights live in the const pool so their (bulky) DMAs can run
    # during the attention stretch when the queues are idle.
    w18 = pool_const.tile([P, CT, HID], fp8, tag="w18", name="w18")
    w28 = pool_const.tile([P, HCT, C], fp8, tag="w28", name="w28")

    def emit_ln(src_tiles, gpp, bpp, dstT, pool_xn, pairs=None, prestats=None,
                xn_dve=False):
        """Token-major LN over C; transpose to feature-major via DMA XBAR
        (two token tiles batched per transpose).

        Wave-structured to avoid per-tile ACT<->DVE ping-pong: all tiles'
        normalization scalars (rstd, nb) are computed first, then the
        normalize+transpose+scale pipeline runs per pair.

        dstT is a single [P, CT, NTOK] bf16 tile: chunk c holds features
        c*128+p on partitions, tokens on the free axis.  prestats, if
        given, maps tile index -> [P, CT, 6] per-chunk bn_stats tile
        (computed earlier, e.g. fused into the residual adds)."""
        ips = list(pairs if pairs is not None else range(NT // 2))
        idxs = [2 * ip + i2 for ip in ips for i2 in range(2)]
        n = len(idxs)
        # batched normalization scalars: aggr per tile into one [P,n,2]
        # buffer, then ONE sqrt / reciprocal / negate-multiply for all tiles
        # (the per-tile versions cost ~0.3us of fixed overhead each)
        mv8 = pool_stats.tile([P, NT, 2], f32, tag="mv8", name="mv8", bufs=2)
        for k, i in enumerate(idxs):
            if prestats is None:
                st6 = pool_stats.tile([P, 6], f32, tag="st6", name="st6")
                nc.vector.bn_stats(st6[:], src_tiles[i][:])
                st6_ap = st6[:]
            else:
                st6_ap = prestats[i][:]
            nc.vector.bn_aggr(mv8[:, k, :], st6_ap)
        sd8 = pool_stats.tile([P, NT], f32, tag="sd8", name="sd8", bufs=2)
        nc.scalar.activation(
            sd8[:, 0:n], mv8[:, 0:n, 1], AF.Sqrt, bias=epsap[:, 0:1]
        )
        rstd8 = pool_stats.tile([P, NT], f32, tag="rstd8", name="rstd8", bufs=2)
        nc.vector.reciprocal(rstd8[:, 0:n], sd8[:, 0:n])
        nb8 = pool_stats.tile([P, NT], f32, tag="nb8", name="nb8", bufs=2)
        nc.vector.scalar_tensor_tensor(
            nb8[:, 0:n], mv8[:, 0:n, 0], -1.0, rstd8[:, 0:n], OP.mult, OP.mult
        )
        rstds = {i: rstd8[:, k : k + 1] for k, i in enumerate(idxs)}
        nbs = {i: nb8[:, k : k + 1] for k, i in enumerate(idxs)}
        for ip in ips:
            xn2 = pool_xn.tile([P, 2, C], bf16, tag="xn", name="xn", bufs=2)
            for i2 in range(2):
                i = 2 * ip + i2
                if xn_dve:
                    # bf16-in/bf16-out runs at the DVE's 2x 16-bit rate and
                    # unloads the ACT during the LN1 startup chain
                    nc.vector.tensor_scalar(
                        xn2[:, i2, :], src_tiles[i][:], rstds[i], nbs[i],
                        OP.mult, OP.add,
                    )
                else:
                    nc.scalar.activation(
                        xn2[:, i2, :], src_tiles[i][:], AF.Identity,
                        bias=nbs[i], scale=rstds[i],
                    )
            # transpose on the PE (idle during LN; ~8x faster than the DMA
            # XBAR): 8 [128,128] blocks into one 1-bank bf16 psum tile
            xr = pool_pbig.tile([P, 2, CT, P], bf16, tag="big", name="xr")
            for i2 in range(2):
                for c in range(CT):
                    nc.tensor.transpose(
                        xr[:, i2, c, :], xn2[:, i2, ts(c, P)], identb[:]
                    )
            for c in range(CT):
                nc.vector.tensor_scalar(
                    dstT[:, c, ts(ip, 2 * P)].rearrange("p (a b) -> p a b", a=2),
                    xr[:, :, c, :],
                    gpp[:, c : c + 1],
                    bpp[:, c : c + 1],
                    OP.mult,
                    OP.add,
                )

    # ================= LN1 + QKV (fp8 DoubleRow) =================
    pool_a = tc.alloc_tile_pool(name="poolA", bufs=1)
    wq8 = pool_a.tile([P, CT, 3 * C], fp8, tag="wq8", name="wq8")
    # scalar queue, behind the second x half
    nc.scalar.dma_start(wq8[:], d["qkv_w8"])
    xlnT = pool_a.tile([P, CT, NTOK], fp8, tag="xlnT", name="xlnT")

    pool_b = tc.alloc_tile_pool(name="poolB", bufs=1, side="right")
    Q_sb = [pool_b.tile([P, NTOK], bf16, tag=f"Q{t}", name=f"Q{t}") for t in range(NP)]
    K_sb = [pool_b.tile([P, NTOK], bf16, tag=f"K{t}", name=f"K{t}") for t in range(NP)]
    # V in fp8, one tile with key-tile-major layout
    V2 = pool_b.tile([P, NT, NH, D], fp8, tag="V2", name="V2")
    # E tiles (fp8, merged per parity so j-pairs sit at a %16 plane stride
    # for DR): head A of each pair gets exact ACT exp, head B gets a
    # Schraudolph fast-exp on the DVE (uint8 bitcast -> fp8e4m3)
    EAb = [pool_b.tile([P, NT, NTOK], fp8, tag=f"EAb{p}", name=f"EAb{p}")
           for p in range(2)]
    EBb = [pool_b.tile([P, NT, NTOK], fp8, tag=f"EBb{p}", name=f"EBb{p}")
           for p in range(2)]

    def emit_qk(nqc):
        # Q/K head-pair chunks (qkv_wT host-reordered [Qpairs|Kpairs|V]):
        # psum rows 0-63 = head 2t, 64-127 = head 2t+1.
        for t in range(NP):
            for dst, base, tg in ((Q_sb, 0, "av"), (K_sb, C, "dn")):
                ps = pool_pav.tile([P, 512], f32, tag=tg, name=tg)
                for cp in range(CT // 2):
                    nc.tensor.matmul(
                        ps[:],
                        wq8[:, 2 * cp : 2 * cp + 2, base + t * P : base + (t + 1) * P],
                        xlnT[:, 2 * cp : 2 * cp + 2, ts(nqc, 512)],
                        start=(cp == 0),
                        stop=(cp == CT // 2 - 1),
                        perf_mode=DR,
                    )
                nc.vector.tensor_scalar(
                    dst[t][:, ts(nqc, 512)], ps[:], 1.0 / W8SCALE, 0.0,
                    OP.mult, OP.add,
                )

    def warm_pe(srcs):
        # staggered dummy matmuls that each wait on a freshly-produced tile
        # chunk: keeps the PE's HAM activity window busy across PE-idle
        # stretches so the real matmuls that follow start at full clock.
        for n, src in enumerate(srcs):
            ps = pool_pav.tile([P, 512], f32, tag=("av", "dn")[n % 2], name="wm")
            nc.tensor.matmul(ps[0:D, 0:64], src, src)

    emit_ln(x_sb, g1pp, b1pp, xlnT, pool_a, xn_dve=True)
    warm_pe([xlnT[:, c, 0:64] for c in (0, 2)])
    emit_qk(0)
    emit_qk(1)
    dump("xlnT", xlnT[:])

    def emit_V(j):
        # V in token-major: [tok, (h, dv)]
        ps = pool_pav.tile([P, 512], f32, tag="av", name="av")
        for cp in range(CT // 2):
            nc.tensor.matmul(
                ps[:],
                xlnT[:, 2 * cp : 2 * cp + 2, ts(j, P)],
                wq8[:, 2 * cp : 2 * cp + 2, 2 * C : 3 * C],
                start=(cp == 0),
                stop=(cp == CT // 2 - 1),
                perf_mode=DR,
            )
        nc.vector.tensor_scalar(
            V2[:, j, :, :], ps[:].rearrange("p (h r) -> p h r", h=NH),
            1.0 / W8SCALE, 0.0, OP.mult, OP.add,
        )

    # ================= attention (software-pipelined head pairs) ==========
    pend = {}

    def avdn_j(t, j):
        """A@V + ones-denominator matmuls for pair t, key tile j (both
        query chunks).  Heads share banks: A rows 0:64, B rows 64:128.
        (Non-DR: M=64 outputs column-tile to ~1.4x rate.)"""
        if t not in pend:
            pend[t] = (
                [pool_pav.tile([P, 512], f32, tag="av", name="av") for _ in range(2)],
                [pool_pav.tile([P, 512], f32, tag="dn", name="dn") for _ in range(2)],
            )
        avs, dns = pend[t]
        st, sp = (j == 0), (j == NT - 1)
        par = t % 2
        for nqc in range(2):
            ea = EAb[par][:, j, ts(nqc, 512)]
            eb = EBb[par][:, j, ts(nqc, 512)]
            av, dn = avs[nqc], dns[nqc]
            nc.tensor.matmul(av[0:D, :], V2[:, j, 2 * t, :], ea, start=st, stop=sp,
                             skip_group_check=True)
            nc.tensor.matmul(av[D : 2 * D, :], V2[:, j, 2 * t + 1, :], eb,
                             start=st, stop=sp, skip_group_check=True)
            nc.tensor.matmul(dn[0:D, :], ones8r[:, 0, :], ea, start=st, stop=sp,
                             skip_group_check=True)
            nc.tensor.matmul(dn[D : 2 * D, :], ones8r[:, 0, :], eb, start=st,
                             stop=sp, skip_group_check=True)

    def finalize(t):
        """Normalize pair t: OT = av / dn (approx-reciprocal + multiply)."""
        avs, dns = pend.pop(t)
        for nqc in range(2):
            dsx = pool_c.tile([P, 512], f32, tag=f"ds{nqc}", name=f"ds{nqc}")
            nc.vector.reciprocal_approx_fast(out=dsx[:], in_=dns[nqc][:])
            nc.vector.tensor_tensor(
                OT2[:, t, ts(nqc, 512)], avs[nqc][:], dsx[:], OP.mult
            )
            if t == 0 and nqc == 1:
                dump("dsA0", dsx[:])
        if t == 0:
            dump("EA0", EAb[0][:, 0, :])

    def scores_j(t, j):
        pbA = pool_pbig.tile([P, NTOK], f32, tag="big", name="big")
        pbB = pool_pbig.tile([P, NTOK], f32, tag="big", name="big")
        for nq in range(2):
            nc.tensor.matmul(
                pbA[:, ts(nq, 512)], K_sb[t][0:D, ts(j, P)],
                Q_sb[t][0:D, ts(nq, 512)],
            )
            nc.tensor.matmul(
                pbB[:, ts(nq, 512)], K_sb[t][D : 2 * D, ts(j, P)],
                Q_sb[t][D : 2 * D, ts(nq, 512)],
            )
        par = t % 2
        nc.scalar.activation(EAb[par][:, j, :], pbA[:], AF.Exp)
        if t % 2 == 1:
            # Schraudolph fast-exp on the DVE for odd pairs' B heads:
            # trunc(alpha*x + beta) as uint8 IS the fp8e4m3 pattern of ~e^x
            # (scores in [-1.3,1.3], far from the uint8 wrap thresholds).
            nc.vector.tensor_scalar(
                EBb[par][:, j, :].bitcast(u8), pbB[:], SCH_ALPHA, SCH_BETA,
                OP.mult, OP.add,
            )
        else:
            nc.scalar.activation(EBb[par][:, j, :], pbB[:], AF.Exp)

    # pair 0: fill the exp-paced gaps with the V projection, then free
    # the LN1/qkv-weight pool before allocating the attention-output pool.
    for j in range(NT):
        scores_j(0, j)
        emit_V(j)
    # bulk weight/zero loads on the sync queue: it is idle for the whole
    # attention stretch, and none of these are needed until proj/MixFFN.
    # (They must not ride the scalar queue -- dma issue occupies the ACT
    # sequencer, which is saturated with softmax exp here.)
    nc.sync.dma_start(w18[:], d["fc1_w8"])
    nc.sync.dma_start(dw8[:], d["dwdiag8"])
    nc.sync.dma_start(w28[:], d["fc2_w8"])
    for t in ht_bufs:
        nc.sync.dma_start(t[:], d["zeros8"])
    pool_a.release()

    pool_c = tc.alloc_tile_pool(name="poolC", bufs=1)
    OT2 = pool_c.tile([P, NP, NTOK], fp8, tag="OT2", name="OT2")
    # fp8 proj weights with pair planes adjacent: DoubleRow halves the
    # contraction passes (output is full-width M=128, so DR forfeits nothing)
    pw8s = pool_c.tile([P, 2, 2, C], fp8, tag="pw8s", name="pw8s")
    nc.sync.dma_start(pw8s[:], d["proj_w8"])
    PT_sb = [pool_c.tile([P, NTOK], bf16, tag=f"PT{c}", name=f"PT{c}") for c in range(CT)]

    for t in range(1, NP):
        for j in range(NT):
            scores_j(t, j)
            avdn_j(t - 1, j)
        finalize(t - 1)
    for j in range(NT):
        avdn_j(NP - 1, j)
    finalize(NP - 1)

    dump("OT0", OT2[:, 0, :])
    pool_b.release()

    # ===== proj + residual 1 + per-chunk LN2 stats (fused per c-chunk) =====
    # proj accumulates in the pav banks (2x[P,512] per oc) so the pbig ring
    # is free for the PE transposes of PT; all proj chains are emitted before
    # the first transpose so the PE never waits on an evac.
    st6_2 = [pool_stats.tile([P, CT, 6], f32, tag=f"st2_{i}", name=f"st2_{i}")
             for i in range(NT)]
    for oc in range(CT):
        for nqc in range(2):
            pq = pool_pav.tile([P, 512], f32, tag=("av", "dn")[nqc], name="pj")
            for m2 in range(2):
                nc.tensor.matmul(
                    pq[:],
                    pw8s[:, m2, :, ts(oc, P)],
                    OT2[:, 2 * m2 : 2 * m2 + 2, ts(nqc, 512)],
                    start=(m2 == 0),
                    stop=(m2 == 1),
                    perf_mode=DR,
                )
            nc.scalar.activation(
                PT_sb[oc][:, ts(nqc, 512)], pq[:], AF.Identity,
                bias=pbpp[:, oc : oc + 1], scale=1.0 / W8SCALE,
            )
    for oc in range(CT):
        ptp = pool_pbig.tile([P, NT, P], bf16, tag="big", name="ptp")
        for i in range(NT):
            nc.tensor.transpose(ptp[:, i, :], PT_sb[oc][:, ts(i, P)], identb[:])
        for i in range(NT):
            nc.vector.tensor_tensor(
                x1_sb[i][:, ts(oc, P)], ptp[:, i, :], x_sb[i][:, ts(oc, P)], OP.add
            )
            nc.vector.bn_stats(st6_2[i][:, oc, :], x1_sb[i][:, ts(oc, P)])

    dump("PT0", PT_sb[0][:])
    pool_c.release()

    # ================= LN2 =================
    pool_d = tc.alloc_tile_pool(name="poolD", bufs=2)
    x2T = pool_d.tile([P, CT, NTOK], fp8, tag="x2T", name="x2T")
    dump("x1_0", x1_sb[0][:])
    warm_pe([x1_sb[2][:, 0:64], x1_sb[5][:, 0:64]])
    emit_ln(x1_sb, g2pp, b2pp, x2T, pool_d, prestats=st6_2)
    warm_pe([x2T[:, c, 0:64] for c in (0, 2)])
    dump("x2T", x2T[:])
    if gelu_mode == "hw":
        # pre-trigger the gelu table set; overlaps the fc1 matmul chains
        nc.scalar.activation(scr1[0:1, :], epsap[0:1, :], AF.Gelu)

    # ================= fc1 + dwconv + gelu (fused per tile) ====
    # PSUM re-plan for the MixFFN: fc1 gets 2x[P,512] (2 banks), the conv
    # psum needs [P,1156] (3 banks) x2 bufs = 6 banks.
    pool_pav.release()
    pool_pbig.release()
    pool_pf = tc.alloc_tile_pool(name="pf", bufs=1, space="PSUM")
    pool_pconv = tc.alloc_tile_pool(name="pconv", bufs=2, space="PSUM")

    pool_e = tc.alloc_tile_pool(name="poolE", bufs=1, side="right")
    Gall = pool_e.tile([P, HCT, NTOK], fp8, tag="Gall", name="Gall")

    def emit_fc1(hc):
        # both query halves into one 2-bank psum tile -> a single wide evac
        ht = ht_bufs[hc % 3]
        ps = pool_pf.tile([P, NTOK], f32, tag="f1", name="f1")
        for nqc in range(2):
            for cp in range(CT // 2):
                nc.tensor.matmul(
                    ps[:, ts(nqc, 512)],
                    w18[:, 2 * cp : 2 * cp + 2, ts(hc, P)],
                    x2T[:, 2 * cp : 2 * cp + 2, ts(nqc, 512)],
                    start=(cp == 0),
                    stop=(cp == CT // 2 - 1),
                    perf_mode=DR,
                )
        # evac on the DVE (ACT is gelu-bound): interior rows 1..32 cols 1..32
        # of plane1's padded grid, one 1024-wide op.
        a0 = CV_G + 34 + 1
        dst = ht[:, 1, a0 : a0 + 32 * 34].rearrange("p (y x) -> p y x", x=34)[
            :, :, 0:32
        ]
        nc.vector.tensor_scalar(
            dst,
            ps[:].rearrange("p (y x) -> p y x", x=WW),
            1.0 / W8SCALE,
            f1bpp[:, hc : hc + 1],
            OP.mult,
            OP.add,
        )
        # shifted copies for the DR tap pairs (near-free on idle DMA queues)
        eng, eng2 = (nc.sync, nc.scalar) if hc % 2 == 0 else (nc.scalar, nc.sync)
        eng.dma_start(
            ht[:, 0, CV_G : CV_G + CV_NPAD], ht[:, 1, CV_G + 2 : CV_G + 2 + CV_NPAD]
        )
        eng2.dma_start(
            ht[:, 2, CV_G : CV_G + CV_NPAD], ht[:, 1, CV_G - 34 : CV_G - 34 + CV_NPAD]
        )

    emit_fc1(0)
    for hc in range(HCT):
        # software pipeline: next chunk's fc1 goes on the PE queue BEFORE this
        # chunk's conv so the PE never waits on the evac+copy chain.
        if hc + 1 < HCT:
            emit_fc1(hc + 1)
        ht = ht_bufs[hc % 3]
        # 5 DR tap-pair matmuls apply all 9 taps (psum covers the full padded
        # grid; garbage lands only in pad positions, never read by the evac).
        # Each is split into 512/512/132 chunks: a matmul output cannot cross
        # a PSUM bank boundary.
        pdc = pool_pconv.tile([P, CV_NPAD], f32, tag="pc", name="pc")
        for n, (q0, x0, _tA, _tB) in enumerate(CV_PAIRS):
            for c0, cn in ((0, 512), (512, 512), (1024, CV_NPAD - 1024)):
                mov = ht[:, q0 : q0 + 2, 0:cn].copy()
                mov.offset = mov.offset + x0 + c0
                nc.tensor.matmul(
                    pdc[:, c0 : c0 + cn],
                    dw8[:, hc, n, :, :],
                    mov,
                    start=(n == 0),
                    stop=(n == len(CV_PAIRS) - 1),
                    perf_mode=DR,
                )
        if hc == 0:
            dump("HT0", ht[:])
        pin = pdc[:, 35 : 35 + 32 * 34].rearrange("p (y x) -> p y x", x=34)[
            :, :, 0:32
        ]
        if gelu_mode == "hw":
            nc.scalar.activation(
                Gall[:, hc, :], pin, AF.Gelu, bias=dwbpp[:, hc : hc + 1],
                scale=1.0 / W8SCALE,
            )
            if hc == 0:
                dump("G0", Gall[:, 0, :])
        else:
            # sim-only fallback: gelu(x) ~= x * sigmoid(1.702 x)
            hb = pool_e.tile([P, NTOK], f32, tag="hb", name="hb", bufs=2)
            nc.scalar.activation(
                hb[:], pin, AF.Identity, bias=dwbpp[:, hc : hc + 1],
                scale=1.0 / W8SCALE,
            )
            sg = pool_e.tile([P, NTOK], f32, tag="sg", name="sg", bufs=2)
            nc.scalar.activation(sg[:], hb[:], AF.Sigmoid, scale=1.702)
            nc.vector.tensor_mul(Gall[:, hc, :], hb[:], sg[:])

    pool_pconv.release()
    pool_pf2 = tc.alloc_tile_pool(name="pf2", bufs=2, space="PSUM")

    # ================= fc2 + residual 2 + output (fused per oc) ===========
    pool_d.release()
    pool_f = tc.alloc_tile_pool(name="poolF", bufs=1)
    FT_sb = [pool_f.tile([P, NTOK], bf16, tag=f"FT{c}", name=f"FT{c}") for c in range(CT)]
    # output partition-major [P, NT, C]: the host transposes back; stores go
    # out in two big-descriptor DMAs instead of 1024 2KB lines
    oall = pool_out.tile([P, NT, C], f32, tag="oall", name="oall")
    ot_sb = [oall[:, i, :] for i in range(NT)]
    for oc in range(CT):
        pq = pool_pf2.tile([P, NTOK], f32, tag="f2", name="f2")
        for nqc in range(2):
            for hp in range(HCT // 2):
                nc.tensor.matmul(
                    pq[:, ts(nqc, 512)],
                    w28[:, 2 * hp : 2 * hp + 2, ts(oc, P)],
                    Gall[:, 2 * hp : 2 * hp + 2, ts(nqc, 512)],
                    start=(hp == 0),
                    stop=(hp == HCT // 2 - 1),
                    perf_mode=DR,
                )
        nc.scalar.activation(
            FT_sb[oc][:], pq[:], AF.Identity, bias=f2bpp[:, oc : oc + 1],
            scale=1.0 / W8SCALE,
        )
        ftp = pool_pf2.tile([P, NT, P], bf16, tag="f2t", name="ftp")
        for i in range(NT):
            nc.tensor.transpose(ftp[:, i, :], FT_sb[oc][:, ts(i, P)], identb[:])
        for i in range(NT):
            nc.vector.tensor_tensor(
                ot_sb[i][:, ts(oc, P)], ftp[:, i, :], x1_sb[i][:, ts(oc, P)], OP.add
            )
            if oc == CT - 1 and i == 5:
                nc.sync.dma_start(out_ap[:, 0:6, :], oall[:, 0:6, :])
            if oc == CT - 1 and i == NT - 1:
                nc.sync.dma_start(out_ap[:, 6:8, :], oall[:, 6:8, :])

    dump("FT0", FT_sb[0][:])
    pool_e.release()
    pool_f.release()
    for p in (pool_pf2, pool_pf, pool_htp, pool_out, pool_stats,
              pool_x1, pool_x, pool_const):
        p.release()


_SHAPES = {
    "biases": (P, 6 * CT + 3 * HCT),
}
_BF16_SHAPES = {
    "xb": (P, NT, C),
}
_FP8_SHAPES = {
    "qkv_w8": (P, CT, 3 * C),
    "fc1_w8": (P, CT, HID),
    "fc2_w8": (P, HCT, C),
    "dwdiag8": (P, HCT, 5, 2, P),
    "zeros8": (P, 3, CV_S),
    "proj_w8": (P, 2, 2, C),
}


DBG_SPECS = {
    "xlnT": ((P, CT, NTOK), "fp8"),
    "EA0": ((P, NTOK), "fp8"),
    "dsA0": ((P, 512), "f32"),
    "OT0": ((P, NTOK), "fp8"),
    "PT0": ((P, NTOK), "bf16"),
    "x1_0": ((P, C), "f32"),
    "x2T": ((P, CT, NTOK), "fp8"),
    "HT0": ((P, 3, CV_S), "fp8"),
    "G0": ((P, NTOK), "fp8"),
    "FT0": ((P, NTOK), "bf16"),
}
_DBG_DT = {"bf16": bf16, "f32": f32, "fp8": fp8}


def build_program(gelu_mode="hw", dbg=False):
    nc = bacc.Bacc(
        "TRN2",
        target_bir_lowering=False,
        debug=False,
        enable_asserts=False,
        num_devices=N_CORES,
    )
    d = {}
    for name, shape in _SHAPES.items():
        d[name] = nc.dram_tensor(name, list(shape), f32, kind="ExternalInput").ap()
    for name, shape in _BF16_SHAPES.items():
        d[name] = nc.dram_tensor(name, list(shape), bf16, kind="ExternalInput").ap()
    for name, shape in _FP8_SHAPES.items():
        d[name] = nc.dram_tensor(name, list(shape), fp8, kind="ExternalInput").ap()
    out_ap = nc.dram_tensor("out", [P, NT, C], f32, kind="ExternalOutput").ap()
    dbg_aps = None
    if dbg:
        dbg_aps = {}
        for k, (shape, dt_) in DBG_SPECS.items():
            dbg_aps[k] = nc.dram_tensor(
                f"dbg_{k}", list(shape), _DBG_DT[dt_],
                kind="ExternalOutput",
            ).ap()
    with tile.TileContext(nc) as tc:
        _emit(tc, d, out_ap, gelu_mode=gelu_mode, dbg=dbg_aps)
    nc.compile()
    return nc


_CACHE = {}
LAST_RESULT = None


def prep_core_inputs(x_b, w):
    """Per-core input map: x_b is this core's [1024, 512] batch slice,
    w the shared host-prepped weight dict."""
    xb = np.ascontiguousarray(
        np.asarray(x_b, np.float32).reshape(NT, P, C).transpose(1, 0, 2)
    ).astype(ml_dtypes.bfloat16)
    m = {"xb": xb}
    m.update(w)
    return m


def prep_weights(inputs):
    qkv_raw = np.asarray(inputs["qkv_w"], np.float32).T  # [C, 3C], head-interleaved
    # reorder columns to [Qpair0..3 | Kpair0..3 | V(head-major)], folding the
    # 1/sqrt(d) score scale into the q columns
    qkv_wT = np.empty((C, 3 * C), np.float32)
    for h in range(NH):
        qcol = qkv_raw[:, h * 3 * D : h * 3 * D + D] * (D ** -0.5)
        kcol = qkv_raw[:, h * 3 * D + D : h * 3 * D + 2 * D]
        vcol = qkv_raw[:, h * 3 * D + 2 * D : h * 3 * D + 3 * D]
        qkv_wT[:, h * D : (h + 1) * D] = qcol
        qkv_wT[:, C + h * D : C + (h + 1) * D] = kcol
        qkv_wT[:, 2 * C + h * D : 2 * C + (h + 1) * D] = vcol
    def pp(v, cols):
        # [cols*P] vector -> [P, cols] per-partition layout
        return np.asarray(v, np.float32).reshape(cols, P).T

    # tap (2,1) per-partition weights (x16 to match the scaled conv psum)
    w9b = np.asarray(inputs["dw_w"], np.float32).reshape(HCT, P, 3, 3)
    dwtap = np.ascontiguousarray(w9b[:, :, 2, 1].T * W8SCALE)  # [P, HCT]
    biases = np.concatenate(
        [
            pp(inputs["ln1_g"], CT), pp(inputs["ln1_b"], CT),
            pp(inputs["ln2_g"], CT), pp(inputs["ln2_b"], CT),
            pp(inputs["proj_b"], CT), pp(inputs["fc2_b"], CT),
            pp(inputs["fc1_b"], HCT), pp(inputs["dw_b"], HCT),
            dwtap,
        ],
        axis=1,
    )
    # fp8 diagonal conv weights for the DR tap-pair matmuls:
    # dwdiag8[k, hc, pair, plane, m] = w[hc*128+k, ky, kx] * W8SCALE if m == k
    w9 = np.asarray(inputs["dw_w"], np.float32).reshape(HCT, P, 3, 3)
    dwdiag8 = np.zeros((P, HCT, 5, 2, P), np.float32)
    kk = np.arange(P)
    for hc in range(HCT):
        for n, (_q0, _x0, tA, tB) in enumerate(CV_PAIRS):
            for q, tap in enumerate((tA, tB)):
                if tap is None:
                    continue
                ky, kx = tap
                dwdiag8[kk, hc, n, q, kk] = w9[hc, :, ky, kx] * W8SCALE
    def to8(wT, nsub):
        # [nsub*P, cols] -> [P, nsub, cols] fp8, scaled up by W8SCALE
        cols = wT.shape[1]
        return np.ascontiguousarray(
            (wT * W8SCALE).reshape(nsub, P, cols).transpose(1, 0, 2)
        ).astype(ml_dtypes.float8_e4m3)

    w = {
        "qkv_w8": to8(qkv_wT, CT),
        "proj_w8": np.ascontiguousarray(
            np.asarray(inputs["proj_w"], np.float32).T.reshape(2, 2, P, C)
            .transpose(2, 0, 1, 3) * W8SCALE
        ).astype(ml_dtypes.float8_e4m3),
        "biases": np.ascontiguousarray(biases),
        "dwdiag8": np.ascontiguousarray(dwdiag8).astype(ml_dtypes.float8_e4m3),
        "zeros8": np.zeros((P, 3, CV_S), ml_dtypes.float8_e4m3),
        "fc1_w8": to8(np.asarray(inputs["fc1_w"], np.float32).T, CT),
        "fc2_w8": to8(np.asarray(inputs["fc2_w"], np.float32).T, HCT),
    }
    return w


def kernel(**inputs):
    x = np.asarray(inputs["x"], np.float32)  # [8, 1024, 512]
    assert x.shape == (N_CORES, NTOK, C), x.shape
    w = prep_weights(inputs)
    if "nc" not in _CACHE:
        _CACHE["nc"] = build_program()
    nc = _CACHE["nc"]
    in_maps = [prep_core_inputs(x[i], w) for i in range(N_CORES)]
    res = bass_utils.run_bass_kernel_spmd(nc, in_maps, core_ids=list(range(N_CORES)))
    global LAST_RESULT
    LAST_RESULT = res
    out = np.stack(
        [
            np.asarray(res.results[i]["out"])
            .transpose(1, 0, 2)
            .reshape(NTOK, C)
            for i in range(N_CORES)
        ],
        axis=0,
    )
    return out.astype(np.float32)



# revision 96
# speedup vs baseline: 1.0058x; 1.0058x over previous
"""Trainium2 Bass kernel for a SegFormer-style transformer block.

Reference computation (per batch element b):
    x  = x + attention(LN1(x))          # 8 heads, d=64, no qkv bias
    x  = x + mixffn(LN2(x))             # fc1 -> dwconv3x3 -> gelu -> fc2

Sharding: pure data-parallel over batch B=8 across the 8 NeuronCores
(one batch element per core, weights replicated, no collectives).

Per-core layout strategy (v3):
  - LayerNorm stats run token-major; the transpose to feature-major goes
    through the DMA XBAR (dma_start_transpose, 16x128 tiles, two token
    tiles per transfer) instead of the PE.
  - Attention processes heads in PAIRS: head 2t lives on partitions 0:64,
    head 2t+1 on 64:128.  Score matmuls (K=64) for the two heads run
    CONCURRENTLY in the PE array via row tile_position (0,0)/(64,0);
    A@V matmuls (M=64) run concurrently via col tile_position (0,0)/(0,64).
    Softmax denominators come from an all-ones [128,64] stationary matmul
    (output = denominator broadcast across 64 partitions, same PSUM bank
    as A@V's pair so the approximate reciprocal runs at base partition 0).
  - The attention is software-pipelined: the exp evacuations pace the
    phase, so pair t's scores interleave with pair t-1's A@V/denominator
    matmuls (pair 0 interleaves with the V projection); E tiles are
    double-buffered across pairs.
  - proj contracts the full 128-dim head pair in one accumulation chain.
  - The depthwise 3x3 conv runs on the PE as 9 diagonal-matrix matmuls
    accumulating in PSUM; diag tiles are built on the DVE in bf16 (4x).
  - PSUM budget (8 banks): pbig 2x[P,1024] + pav 4x[P,512]; matmul
    chains pair both query chunks into one pbig tile so every ACT/DVE
    evacuation is a single 1024-wide instruction (ACT has ~352 cycles of
    fixed overhead per instruction).
  - All matmuls in bf16 (fp32 PSUM accumulation).

Self-contained: hardcodes all shapes; takes full inputs, returns full
output.
"""

import numpy as np
import ml_dtypes

import concourse.bass as bass
import concourse.tile as tile
from concourse import bacc, mybir
from concourse import bass_utils
from concourse.bass import ts, ds
from concourse.masks import make_identity

P = 128
NTOK = 1024
C = 512
HID = 2048
NH = 8
NP = NH // 2        # head pairs
D = 64
HH = 32
WW = 32
NT = NTOK // P      # 8 token tiles
CT = C // P         # 4 feature tiles
HCT = HID // P      # 16 hidden tiles
EPS = 1e-5
N_CORES = 8

f32 = mybir.dt.float32
bf16 = mybir.dt.bfloat16
fp8 = mybir.dt.float8e4
u8 = mybir.dt.uint8
# Schraudolph fast-exp constants: trunc(x*8/ln2 + 56.16) as uint8 bits IS
# fp8e4m3(~e^x); beta tuned for min max log-ratio error under truncation
SCH_ALPHA = 8.0 / 0.6931471805599453
SCH_BETA = 56.16
AF = mybir.ActivationFunctionType
OP = mybir.AluOpType
DR = mybir.MatmulPerfMode.DoubleRow
W8SCALE = 16.0  # fp8 weights are scaled up x16 (host) and back /16 at evac

# ---- depthwise conv v2: fp8 DoubleRow tap-pair matmuls ----
# ht tile layout (per hidden chunk): [P, 3 planes, CV_S] fp8.
#   plane1 = original zero-padded 34x34 grid at byte CV_G (written by the
#            fc1 evacuation, interior rows/cols 1..32 only; pads stay zero
#            from the one-time whole-tile memset).
#   plane0 = plane1 shifted by +2 elements (DMA copy), plane2 = shifted -34.
# A DR matmul reads two planes at the same in-plane offset x0 (middle-dim
# stride CV_S must be %16==0), so one matmul applies TWO taps:
#   via plane1 (orig):    tap delta = x0 - CV_G
#   via plane0 (shift+2): tap delta = x0 - CV_G + 2
#   via plane2 (shift-34): tap delta = x0 - CV_G - 34
# tap (ky,kx) -> delta = (ky-1)*34 + (kx-1) on the padded grid.
CV_S = 1232            # plane stride (fp8 bytes), 77*16
CV_G = 64              # grid origin within a plane (apron before it)
CV_NPAD = 34 * 34      # padded grid size (1156)
# (first_ap_plane, x0, tap_on_q0, tap_on_q1); taps as (ky,kx), None = zeros
CV_PAIRS = [
    (0, 29, (0, 2), (0, 0)),   # deltas -33, -35
    (1, 64, (1, 1), (0, 1)),   # planes (1,2): deltas 0, -34
    (0, 63, (1, 2), (1, 0)),   # deltas +1, -1
    (0, 97, (2, 2), (2, 0)),   # deltas +35, +33
    (0, 98, None, (2, 1)),     # deltas (+36 x zeros), +34
]


def _emit(tc, d, out_ap, gelu_mode="hw", dbg=None):
    def dump(key, ap):
        if dbg is not None and key in dbg:
            tc.nc.sync.dma_start(dbg[key], ap)

    nc = tc.nc

    # ---- whole-kernel pools ----
    pool_const = tc.alloc_tile_pool(name="const", bufs=1)
    pool_x = tc.alloc_tile_pool(name="x", bufs=1)
    pool_x1 = tc.alloc_tile_pool(name="x1", bufs=1)
    pool_stats = tc.alloc_tile_pool(name="stats", bufs=4)
    pool_out = tc.alloc_tile_pool(name="outp", bufs=1)
    # PSUM budget (8 banks): pbig 2x[P,1024] + pav (av,dn) x2 [P,512]
    pool_pbig = tc.alloc_tile_pool(name="pbig", bufs=2, space="PSUM")
    pool_pav = tc.alloc_tile_pool(name="pav", bufs=2, space="PSUM")

    identb = pool_const.tile([P, P], bf16, tag="identb", name="identb")
    make_identity(nc, identb[:])
    # fp8 all-ones stationary for the denominator matmuls
    ones8r = pool_const.tile([P, 2, D], fp8, tag="ones8r", name="ones8r")
    nc.vector.memset(ones8r[:], 1.0)
    zconst = pool_const.tile([P, 1], f32, tag="zconst", name="zconst")
    nc.vector.memset(zconst[:], 0.0)
    nc.const_aps.aps[(f32, 0.0)] = zconst[:]
    epsap = pool_const.tile([P, 1], f32, tag="epsap", name="epsap")
    nc.vector.memset(epsap[:], EPS)

    # all per-partition bias vectors arrive host-packed in one [P, 56] DMA
    # (small, ahead of the x tiles on the sync queue)
    bias_sb = pool_const.tile([P, 6 * CT + 3 * HCT], f32, tag="biases", name="biases")
    nc.sync.dma_start(bias_sb[:], d["biases"])
    g1pp = bias_sb[:, 0:4]
    b1pp = bias_sb[:, 4:8]
    g2pp = bias_sb[:, 8:12]
    b2pp = bias_sb[:, 12:16]
    pbpp = bias_sb[:, 16:20]
    f2bpp = bias_sb[:, 20:24]
    f1bpp = bias_sb[:, 24:40]
    dwbpp = bias_sb[:, 40:56]
    dwtpp = bias_sb[:, 56:72]
    # host-prebuilt fp8 diagonal conv weights: [P, hc, pair, plane, 128].
    # The tile lives in the const pool but its (bulky) DMA is emitted later,
    # once the latency-critical LN1 transfers have cleared the queues.
    dw8 = pool_const.tile([P, HCT, 5, 2, P], fp8, tag="dw8", name="dw8")
    # persistent triple-buffered conv input tiles; zero-filled once via DMA
    # (engine-free) so pads/aprons/tails read as zeros forever (the fc1 evac
    # only writes interiors, the plane copies rewrite full grids).
    pool_htp = tc.alloc_tile_pool(name="htp", bufs=1, side="right")
    ht_bufs = [
        pool_htp.tile([P, 3, CV_S], fp8, tag=f"htb{i}", name=f"htb{i}")
        for i in range(3)
    ]
    # pre-trigger the sqrt table set (LN rstd) while the input DMAs run
    scr1 = pool_const.tile([P, 1], f32, tag="scr1", name="scr1")
    nc.scalar.activation(scr1[0:1, :], epsap[0:1, :], AF.Sqrt)

    # x arrives host-repacked partition-major bf16 [P, NT, C]: two DMAs with
    # 8KB-contiguous partition lines instead of 1024 2KB descriptors.
    xall = pool_x.tile([P, NT, C], bf16, tag="xall", name="xall")
    nc.sync.dma_start(xall[:, 0:1, :], d["xb"][:, 0:1, :])
    nc.sync.dma_start(xall[:, 1:4, :], d["xb"][:, 1:4, :])
    nc.scalar.dma_start(xall[:, 4:8, :], d["xb"][:, 4:8, :])
    x_sb = [xall[:, i, :] for i in range(NT)]
    x1_sb = [pool_x1.tile([P, C], f32, tag=f"x1_{i}", name=f"x1_{i}") for i in range(NT)]
    # fc1/fc2 weights live in the const pool so their (bulky) DMAs can run
    # during the attention stretch when the queues are idle.
    w18 = pool_const.tile([P, CT, HID], fp8, tag="w18", name="w18")
    w28 = pool_const.tile([P, HCT, C], fp8, tag="w28", name="w28")

    def emit_ln(src_tiles, gpp, bpp, dstT, pool_xn, pairs=None, prestats=None):
        """Token-major LN over C; transpose to feature-major via DMA XBAR
        (two token tiles batched per transpose).

        Wave-structured to avoid per-tile ACT<->DVE ping-pong: all tiles'
        normalization scalars (rstd, nb) are computed first, then the
        normalize+transpose+scale pipeline runs per pair.

        dstT is a single [P, CT, NTOK] bf16 tile: chunk c holds features
        c*128+p on partitions, tokens on the free axis.  prestats, if
        given, maps tile index -> [P, CT, 6] per-chunk bn_stats tile
        (computed earlier, e.g. fused into the residual adds)."""
        ips = list(pairs if pairs is not None else range(NT // 2))
        idxs = [2 * ip + i2 for ip in ips for i2 in range(2)]
        n = len(idxs)
        # batched normalization scalars: aggr per tile into one [P,n,2]
        # buffer, then ONE sqrt / reciprocal / negate-multiply for all tiles
        # (the per-tile versions cost ~0.3us of fixed overhead each)
        mv8 = pool_stats.tile([P, NT, 2], f32, tag="mv8", name="mv8", bufs=2)
        for k, i in enumerate(idxs):
            if prestats is None:
                st6 = pool_stats.tile([P, 6], f32, tag="st6", name="st6")
                nc.vector.bn_stats(st6[:], src_tiles[i][:])
                st6_ap = st6[:]
            else:
                st6_ap = prestats[i][:]
            nc.vector.bn_aggr(mv8[:, k, :], st6_ap)
        sd8 = pool_stats.tile([P, NT], f32, tag="sd8", name="sd8", bufs=2)
        nc.scalar.activation(
            sd8[:, 0:n], mv8[:, 0:n, 1], AF.Sqrt, bias=epsap[:, 0:1]
        )
        rstd8 = pool_stats.tile([P, NT], f32, tag="rstd8", name="rstd8", bufs=2)
        nc.vector.reciprocal(rstd8[:, 0:n], sd8[:, 0:n])
        nb8 = pool_stats.tile([P, NT], f32, tag="nb8", name="nb8", bufs=2)
        nc.vector.scalar_tensor_tensor(
            nb8[:, 0:n], mv8[:, 0:n, 0], -1.0, rstd8[:, 0:n], OP.mult, OP.mult
        )
        rstds = {i: rstd8[:, k : k + 1] for k, i in enumerate(idxs)}
        nbs = {i: nb8[:, k : k + 1] for k, i in enumerate(idxs)}
        for ip in ips:
            xn2 = pool_xn.tile([P, 2, C], bf16, tag="xn", name="xn", bufs=2)
            for i2 in range(2):
                i = 2 * ip + i2
                nc.scalar.activation(
                    xn2[:, i2, :], src_tiles[i][:], AF.Identity,
                    bias=nbs[i], scale=rstds[i],
                )
            # transpose on the PE (idle during LN; ~8x faster than the DMA
            # XBAR): 8 [128,128] blocks into one 1-bank bf16 psum tile
            xr = pool_pbig.tile([P, 2, CT, P], bf16, tag="big", name="xr")
            for i2 in range(2):
                for c in range(CT):
                    nc.tensor.transpose(
                        xr[:, i2, c, :], xn2[:, i2, ts(c, P)], identb[:]
                    )
            for c in range(CT):
                nc.vector.tensor_scalar(
                    dstT[:, c, ts(ip, 2 * P)].rearrange("p (a b) -> p a b", a=2),
                    xr[:, :, c, :],
                    gpp[:, c : c + 1],
                    bpp[:, c : c + 1],
                    OP.mult,
                    OP.add,
                )

    # ================= LN1 + QKV (fp8 DoubleRow) =================
    pool_a = tc.alloc_tile_pool(name="poolA", bufs=1)
    wq8 = pool_a.tile([P, CT, 3 * C], fp8, tag="wq8", name="wq8")
    # scalar queue, behind the second x half
    nc.scalar.dma_start(wq8[:], d["qkv_w8"])
    xlnT = pool_a.tile([P, CT, NTOK], fp8, tag="xlnT", name="xlnT")

    pool_b = tc.alloc_tile_pool(name="poolB", bufs=1, side="right")
    Q_sb = [pool_b.tile([P, NTOK], bf16, tag=f"Q{t}", name=f"Q{t}") for t in range(NP)]
    K_sb = [pool_b.tile([P, NTOK], bf16, tag=f"K{t}", name=f"K{t}") for t in range(NP)]
    # V in fp8, one tile with key-tile-major layout
    V2 = pool_b.tile([P, NT, NH, D], fp8, tag="V2", name="V2")
    # E tiles (fp8, merged per parity so j-pairs sit at a %16 plane stride
    # for DR): head A of each pair gets exact ACT exp, head B gets a
    # Schraudolph fast-exp on the DVE (uint8 bitcast -> fp8e4m3)
    EAb = [pool_b.tile([P, NT, NTOK], fp8, tag=f"EAb{p}", name=f"EAb{p}")
           for p in range(2)]
    EBb = [pool_b.tile([P, NT, NTOK], fp8, tag=f"EBb{p}", name=f"EBb{p}")
           for p in range(2)]

    def emit_qk(nqc):
        # Q/K head-pair chunks (qkv_wT host-reordered [Qpairs|Kpairs|V]):
        # psum rows 0-63 = head 2t, 64-127 = head 2t+1.
        for t in range(NP):
            for dst, base, tg in ((Q_sb, 0, "av"), (K_sb, C, "dn")):
                ps = pool_pav.tile([P, 512], f32, tag=tg, name=tg)
                for cp in range(CT // 2):
                    nc.tensor.matmul(
                        ps[:],
                        wq8[:, 2 * cp : 2 * cp + 2, base + t * P : base + (t + 1) * P],
                        xlnT[:, 2 * cp : 2 * cp + 2, ts(nqc, 512)],
                        start=(cp == 0),
                        stop=(cp == CT // 2 - 1),
                        perf_mode=DR,
                    )
                nc.vector.tensor_scalar(
                    dst[t][:, ts(nqc, 512)], ps[:], 1.0 / W8SCALE, 0.0,
                    OP.mult, OP.add,
                )

    def warm_pe(srcs):
        # staggered dummy matmuls that each wait on a freshly-produced tile
        # chunk: keeps the PE's HAM activity window busy across PE-idle
        # stretches so the real matmuls that follow start at full clock.
        for n, src in enumerate(srcs):
            ps = pool_pav.tile([P, 512], f32, tag=("av", "dn")[n % 2], name="wm")
            nc.tensor.matmul(ps[0:D, 0:64], src, src)

    emit_ln(x_sb, g1pp, b1pp, xlnT, pool_a)
    warm_pe([xlnT[:, c, 0:64] for c in (0, 2)])
    emit_qk(0)
    emit_qk(1)
    dump("xlnT", xlnT[:])

    def emit_V(j):
        # V in token-major: [tok, (h, dv)]
        ps = pool_pav.tile([P, 512], f32, tag="av", name="av")
        for cp in range(CT // 2):
            nc.tensor.matmul(
                ps[:],
                xlnT[:, 2 * cp : 2 * cp + 2, ts(j, P)],
                wq8[:, 2 * cp : 2 * cp + 2, 2 * C : 3 * C],
                start=(cp == 0),
                stop=(cp == CT // 2 - 1),
                perf_mode=DR,
            )
        nc.vector.tensor_scalar(
            V2[:, j, :, :], ps[:].rearrange("p (h r) -> p h r", h=NH),
            1.0 / W8SCALE, 0.0, OP.mult, OP.add,
        )

    # ================= attention (software-pipelined head pairs) ==========
    pend = {}

    def avdn_j(t, j):
        """A@V + ones-denominator matmuls for pair t, key tile j (both
        query chunks).  Heads share banks: A rows 0:64, B rows 64:128.
        (Non-DR: M=64 outputs column-tile to ~1.4x rate.)"""
        if t not in pend:
            pend[t] = (
                [pool_pav.tile([P, 512], f32, tag="av", name="av") for _ in range(2)],
                [pool_pav.tile([P, 512], f32, tag="dn", name="dn") for _ in range(2)],
            )
        avs, dns = pend[t]
        st, sp = (j == 0), (j == NT - 1)
        par = t % 2
        for nqc in range(2):
            ea = EAb[par][:, j, ts(nqc, 512)]
            eb = EBb[par][:, j, ts(nqc, 512)]
            av, dn = avs[nqc], dns[nqc]
            nc.tensor.matmul(av[0:D, :], V2[:, j, 2 * t, :], ea, start=st, stop=sp,
                             skip_group_check=True)
            nc.tensor.matmul(av[D : 2 * D, :], V2[:, j, 2 * t + 1, :], eb,
                             start=st, stop=sp, skip_group_check=True)
            nc.tensor.matmul(dn[0:D, :], ones8r[:, 0, :], ea, start=st, stop=sp,
                             skip_group_check=True)
            nc.tensor.matmul(dn[D : 2 * D, :], ones8r[:, 0, :], eb, start=st,
                             stop=sp, skip_group_check=True)

    def finalize(t):
        """Normalize pair t: OT = av / dn (approx-reciprocal + multiply)."""
        avs, dns = pend.pop(t)
        for nqc in range(2):
            dsx = pool_c.tile([P, 512], f32, tag=f"ds{nqc}", name=f"ds{nqc}")
            nc.vector.reciprocal_approx_fast(out=dsx[:], in_=dns[nqc][:])
            nc.vector.tensor_tensor(
                OT2[:, t, ts(nqc, 512)], avs[nqc][:], dsx[:], OP.mult
            )
            if t == 0 and nqc == 1:
                dump("dsA0", dsx[:])
        if t == 0:
            dump("EA0", EAb[0][:, 0, :])

    def scores_j(t, j):
        pbA = pool_pbig.tile([P, NTOK], f32, tag="big", name="big")
        pbB = pool_pbig.tile([P, NTOK], f32, tag="big", name="big")
        for nq in range(2):
            nc.tensor.matmul(
                pbA[:, ts(nq, 512)], K_sb[t][0:D, ts(j, P)],
                Q_sb[t][0:D, ts(nq, 512)],
            )
            nc.tensor.matmul(
                pbB[:, ts(nq, 512)], K_sb[t][D : 2 * D, ts(j, P)],
                Q_sb[t][D : 2 * D, ts(nq, 512)],
            )
        par = t % 2
        nc.scalar.activation(EAb[par][:, j, :], pbA[:], AF.Exp)
        if t % 2 == 1:
            # Schraudolph fast-exp on the DVE for odd pairs' B heads:
            # trunc(alpha*x + beta) as uint8 IS the fp8e4m3 pattern of ~e^x
            # (scores in [-1.3,1.3], far from the uint8 wrap thresholds).
            nc.vector.tensor_scalar(
                EBb[par][:, j, :].bitcast(u8), pbB[:], SCH_ALPHA, SCH_BETA,
                OP.mult, OP.add,
            )
        else:
            nc.scalar.activation(EBb[par][:, j, :], pbB[:], AF.Exp)

    # pair 0: fill the exp-paced gaps with the V projection, then free
    # the LN1/qkv-weight pool before allocating the attention-output pool.
    for j in range(NT):
        scores_j(0, j)
        emit_V(j)
    # bulk weight/zero loads on the sync queue: it is idle for the whole
    # attention stretch, and none of these are needed until proj/MixFFN.
    # (They must not ride the scalar queue -- dma issue occupies the ACT
    # sequencer, which is saturated with softmax exp here.)
    nc.sync.dma_start(w18[:], d["fc1_w8"])
    nc.sync.dma_start(dw8[:], d["dwdiag8"])
    nc.sync.dma_start(w28[:], d["fc2_w8"])
    for t in ht_bufs:
        nc.sync.dma_start(t[:], d["zeros8"])
    pool_a.release()

    pool_c = tc.alloc_tile_pool(name="poolC", bufs=1)
    OT2 = pool_c.tile([P, NP, NTOK], fp8, tag="OT2", name="OT2")
    # fp8 proj weights with pair planes adjacent: DoubleRow halves the
    # contraction passes (output is full-width M=128, so DR forfeits nothing)
    pw8s = pool_c.tile([P, 2, 2, C], fp8, tag="pw8s", name="pw8s")
    nc.sync.dma_start(pw8s[:], d["proj_w8"])
    PT_sb = [pool_c.tile([P, NTOK], bf16, tag=f"PT{c}", name=f"PT{c}") for c in range(CT)]

    for t in range(1, NP):
        for j in range(NT):
            scores_j(t, j)
            avdn_j(t - 1, j)
        finalize(t - 1)
    for j in range(NT):
        avdn_j(NP - 1, j)
    finalize(NP - 1)

    dump("OT0", OT2[:, 0, :])
    pool_b.release()

    # ===== proj + residual 1 + per-chunk LN2 stats (fused per c-chunk) =====
    # proj accumulates in the pav banks (2x[P,512] per oc) so the pbig ring
    # is free for the PE transposes of PT; all proj chains are emitted before
    # the first transpose so the PE never waits on an evac.
    st6_2 = [pool_stats.tile([P, CT, 6], f32, tag=f"st2_{i}", name=f"st2_{i}")
             for i in range(NT)]
    for oc in range(CT):
        for nqc in range(2):
            pq = pool_pav.tile([P, 512], f32, tag=("av", "dn")[nqc], name="pj")
            for m2 in range(2):
                nc.tensor.matmul(
                    pq[:],
                    pw8s[:, m2, :, ts(oc, P)],
                    OT2[:, 2 * m2 : 2 * m2 + 2, ts(nqc, 512)],
                    start=(m2 == 0),
                    stop=(m2 == 1),
                    perf_mode=DR,
                )
            nc.scalar.activation(
                PT_sb[oc][:, ts(nqc, 512)], pq[:], AF.Identity,
                bias=pbpp[:, oc : oc + 1], scale=1.0 / W8SCALE,
            )
    for oc in range(CT):
        ptp = pool_pbig.tile([P, NT, P], bf16, tag="big", name="ptp")
        for i in range(NT):
            nc.tensor.transpose(ptp[:, i, :], PT_sb[oc][:, ts(i, P)], identb[:])
        for i in range(NT):
            nc.vector.tensor_tensor(
                x1_sb[i][:, ts(oc, P)], ptp[:, i, :], x_sb[i][:, ts(oc, P)], OP.add
            )
            nc.vector.bn_stats(st6_2[i][:, oc, :], x1_sb[i][:, ts(oc, P)])

    dump("PT0", PT_sb[0][:])
    pool_c.release()

    # ================= LN2 =================
    pool_d = tc.alloc_tile_pool(name="poolD", bufs=2)
    x2T = pool_d.tile([P, CT, NTOK], fp8, tag="x2T", name="x2T")
    dump("x1_0", x1_sb[0][:])
    warm_pe([x1_sb[2][:, 0:64], x1_sb[5][:, 0:64]])
    emit_ln(x1_sb, g2pp, b2pp, x2T, pool_d, prestats=st6_2)
    warm_pe([x2T[:, c, 0:64] for c in (0, 2)])
    dump("x2T", x2T[:])
    if gelu_mode == "hw":
        # pre-trigger the gelu table set; overlaps the fc1 matmul chains
        nc.scalar.activation(scr1[0:1, :], epsap[0:1, :], AF.Gelu)

    # ================= fc1 + dwconv + gelu (fused per tile) ====
    # PSUM re-plan for the MixFFN: fc1 gets 2x[P,512] (2 banks), the conv
    # psum needs [P,1156] (3 banks) x2 bufs = 6 banks.
    pool_pav.release()
    pool_pbig.release()
    pool_pf = tc.alloc_tile_pool(name="pf", bufs=1, space="PSUM")
    pool_pconv = tc.alloc_tile_pool(name="pconv", bufs=2, space="PSUM")

    pool_e = tc.alloc_tile_pool(name="poolE", bufs=1, side="right")
    Gall = pool_e.tile([P, HCT, NTOK], fp8, tag="Gall", name="Gall")

    def emit_fc1(hc):
        # both query halves into one 2-bank psum tile -> a single wide evac
        ht = ht_bufs[hc % 3]
        ps = pool_pf.tile([P, NTOK], f32, tag="f1", name="f1")
        for nqc in range(2):
            for cp in range(CT // 2):
                nc.tensor.matmul(
                    ps[:, ts(nqc, 512)],
                    w18[:, 2 * cp : 2 * cp + 2, ts(hc, P)],
                    x2T[:, 2 * cp : 2 * cp + 2, ts(nqc, 512)],
                    start=(cp == 0),
                    stop=(cp == CT // 2 - 1),
                    perf_mode=DR,
                )
        # evac on the DVE (ACT is gelu-bound): interior rows 1..32 cols 1..32
        # of plane1's padded grid, one 1024-wide op.
        a0 = CV_G + 34 + 1
        dst = ht[:, 1, a0 : a0 + 32 * 34].rearrange("p (y x) -> p y x", x=34)[
            :, :, 0:32
        ]
        nc.vector.tensor_scalar(
            dst,
            ps[:].rearrange("p (y x) -> p y x", x=WW),
            1.0 / W8SCALE,
            f1bpp[:, hc : hc + 1],
            OP.mult,
            OP.add,
        )
        # shifted copies for the DR tap pairs (near-free on idle DMA queues)
        eng, eng2 = (nc.sync, nc.scalar) if hc % 2 == 0 else (nc.scalar, nc.sync)
        eng.dma_start(
            ht[:, 0, CV_G : CV_G + CV_NPAD], ht[:, 1, CV_G + 2 : CV_G + 2 + CV_NPAD]
        )
        eng2.dma_start(
            ht[:, 2, CV_G : CV_G + CV_NPAD], ht[:, 1, CV_G - 34 : CV_G - 34 + CV_NPAD]
        )

    emit_fc1(0)
    for hc in range(HCT):
        # software pipeline: next chunk's fc1 goes on the PE queue BEFORE this
        # chunk's conv so the PE never waits on the evac+copy chain.
        if hc + 1 < HCT:
            emit_fc1(hc + 1)
        ht = ht_bufs[hc % 3]
        # 5 DR tap-pair matmuls apply all 9 taps (psum covers the full padded
        # grid; garbage lands only in pad positions, never read by the evac).
        # Each is split into 512/512/132 chunks: a matmul output cannot cross
        # a PSUM bank boundary.
        pdc = pool_pconv.tile([P, CV_NPAD], f32, tag="pc", name="pc")
        for n, (q0, x0, _tA, _tB) in enumerate(CV_PAIRS):
            for c0, cn in ((0, 512), (512, 512), (1024, CV_NPAD - 1024)):
                mov = ht[:, q0 : q0 + 2, 0:cn].copy()
                mov.offset = mov.offset + x0 + c0
                nc.tensor.matmul(
                    pdc[:, c0 : c0 + cn],
                    dw8[:, hc, n, :, :],
                    mov,
                    start=(n == 0),
                    stop=(n == len(CV_PAIRS) - 1),
                    perf_mode=DR,
                )
        if hc == 0:
            dump("HT0", ht[:])
        pin = pdc[:, 35 : 35 + 32 * 34].rearrange("p (y x) -> p y x", x=34)[
            :, :, 0:32
        ]
        if gelu_mode == "hw":
            nc.scalar.activation(
                Gall[:, hc, :], pin, AF.Gelu, bias=dwbpp[:, hc : hc + 1],
                scale=1.0 / W8SCALE,
            )
            if hc == 0:
                dump("G0", Gall[:, 0, :])
        else:
            # sim-only fallback: gelu(x) ~= x * sigmoid(1.702 x)
            hb = pool_e.tile([P, NTOK], f32, tag="hb", name="hb", bufs=2)
            nc.scalar.activation(
                hb[:], pin, AF.Identity, bias=dwbpp[:, hc : hc + 1],
                scale=1.0 / W8SCALE,
            )
            sg = pool_e.tile([P, NTOK], f32, tag="sg", name="sg", bufs=2)
            nc.scalar.activation(sg[:], hb[:], AF.Sigmoid, scale=1.702)
            nc.vector.tensor_mul(Gall[:, hc, :], hb[:], sg[:])

    pool_pconv.release()
    pool_pf2 = tc.alloc_tile_pool(name="pf2", bufs=2, space="PSUM")

    # ================= fc2 + residual 2 + output (fused per oc) ===========
    pool_d.release()
    pool_f = tc.alloc_tile_pool(name="poolF", bufs=1)
    FT_sb = [pool_f.tile([P, NTOK], bf16, tag=f"FT{c}", name=f"FT{c}") for c in range(CT)]
    # output partition-major [P, NT, C]: the host transposes back; stores go
    # out in two big-descriptor DMAs instead of 1024 2KB lines
    oall = pool_out.tile([P, NT, C], f32, tag="oall", name="oall")
    ot_sb = [oall[:, i, :] for i in range(NT)]
    for oc in range(CT):
        pq = pool_pf2.tile([P, NTOK], f32, tag="f2", name="f2")
        for nqc in range(2):
            for hp in range(HCT // 2):
                nc.tensor.matmul(
                    pq[:, ts(nqc, 512)],
                    w28[:, 2 * hp : 2 * hp + 2, ts(oc, P)],
                    Gall[:, 2 * hp : 2 * hp + 2, ts(nqc, 512)],
                    start=(hp == 0),
                    stop=(hp == HCT // 2 - 1),
                    perf_mode=DR,
                )
        nc.scalar.activation(
            FT_sb[oc][:], pq[:], AF.Identity, bias=f2bpp[:, oc : oc + 1],
            scale=1.0 / W8SCALE,
        )
        ftp = pool_pf2.tile([P, NT, P], bf16, tag="f2t", name="ftp")
        for i in range(NT):
            nc.tensor.transpose(ftp[:, i, :], FT_sb[oc][:, ts(i, P)], identb[:])
        for i in range(NT):
            nc.vector.tensor_tensor(
                ot_sb[i][:, ts(oc, P)], ftp[:, i, :], x1_sb[i][:, ts(oc, P)], OP.add
            )
            if oc == CT - 1 and i == 5:
                nc.sync.dma_start(out_ap[:, 0:6, :], oall[:, 0:6, :])
            if oc == CT - 1 and i == NT - 1:
                nc.sync.dma_start(out_ap[:, 6:8, :], oall[:, 6:8, :])

    dump("FT0", FT_sb[0][:])
    pool_e.release()
    pool_f.release()
    for p in (pool_pf2, pool_pf, pool_htp, pool_out, pool_stats,
              pool_x1, pool_x, pool_const):
        p.release()


_SHAPES = {
    "biases": (P, 6 * CT + 3 * HCT),
}
_BF16_SHAPES = {
    "xb": (P, NT, C),
}
_FP8_SHAPES = {
    "qkv_w8": (P, CT, 3 * C),
    "fc1_w8": (P, CT, HID),
    "fc2_w8": (P, HCT, C),
    "dwdiag8": (P, HCT, 5, 2, P),
    "zeros8": (P, 3, CV_S),
    "proj_w8": (P, 2, 2, C),
}


DBG_SPECS = {
    "xlnT": ((P, CT, NTOK), "fp8"),
    "EA0": ((P, NTOK), "fp8"),
    "dsA0": ((P, 512), "f32"),
    "OT0": ((P, NTOK), "fp8"),
    "PT0": ((P, NTOK), "bf16"),
    "x1_0": ((P, C), "f32"),
    "x2T": ((P, CT, NTOK), "fp8"),
    "HT0": ((P, 3, CV_S), "fp8"),
    "G0": ((P, NTOK), "fp8"),
    "FT0": ((P, NTOK), "bf16"),
}
_DBG_DT = {"bf16": bf16, "f32": f32, "fp8": fp8}


def build_program(gelu_mode="hw", dbg=False):
    nc = bacc.Bacc(
        "TRN2",
        target_bir_lowering=False,
        debug=False,
        enable_asserts=False,
        num_devices=N_CORES,
    )
    d = {}
    for name, shape in _SHAPES.items():
        d[name] = nc.dram_tensor(name, list(shape), f32, kind="ExternalInput").ap()
    for name, shape in _BF16_SHAPES.items():
        d[name] = nc.dram_tensor(name, list(shape), bf16, kind="ExternalInput").ap()
    for name, shape in _FP8_SHAPES.items():
        d[name] = nc.dram_tensor(name, list(shape), fp8, kind="ExternalInput").ap()
    out_ap = nc.dram_tensor("out", [P, NT, C], f32, kind="ExternalOutput").ap()
    dbg_aps = None
    if dbg:
        dbg_aps = {}
        for k, (shape, dt_) in DBG_SPECS.items():
            dbg_aps[k] = nc.dram_tensor(
                f"dbg_{k}", list(shape), _DBG_DT[dt_],
                kind="ExternalOutput",
            ).ap()
    with tile.TileContext(nc) as tc:
        _emit(tc, d, out_ap, gelu_mode=gelu_mode, dbg=dbg_aps)
    nc.compile()
    return nc


_CACHE = {}
LAST_RESULT = None


def prep_core_inputs(x_b, w):
    """Per-core input map: x_b is this core's [1024, 512] batch slice,
    w the shared host-prepped weight dict."""
    xb = np.ascontiguousarray(
        np.asarray(x_b, np.float32).reshape(NT, P, C).transpose(1, 0, 2)
    ).astype(ml_dtypes.bfloat16)
    m = {"xb": xb}
    m.update(w)
    return m


def prep_weights(inputs):
    qkv_raw = np.asarray(inputs["qkv_w"], np.float32).T  # [C, 3C], head-interleaved
    # reorder columns to [Qpair0..3 | Kpair0..3 | V(head-major)], folding the
    # 1/sqrt(d) score scale into the q columns
    qkv_wT = np.empty((C, 3 * C), np.float32)
    for h in range(NH):
        qcol = qkv_raw[:, h * 3 * D : h * 3 * D + D] * (D ** -0.5)
        kcol = qkv_raw[:, h * 3 * D + D : h * 3 * D + 2 * D]
        vcol = qkv_raw[:, h * 3 * D + 2 * D : h * 3 * D + 3 * D]
        qkv_wT[:, h * D : (h + 1) * D] = qcol
        qkv_wT[:, C + h * D : C + (h + 1) * D] = kcol
        qkv_wT[:, 2 * C + h * D : 2 * C + (h + 1) * D] = vcol
    def pp(v, cols):
        # [cols*P] vector -> [P, cols] per-partition layout
        return np.asarray(v, np.float32).reshape(cols, P).T

    # tap (2,1) per-partition weights (x16 to match the scaled conv psum)
    w9b = np.asarray(inputs["dw_w"], np.float32).reshape(HCT, P, 3, 3)
    dwtap = np.ascontiguousarray(w9b[:, :, 2, 1].T * W8SCALE)  # [P, HCT]
    biases = np.concatenate(
        [
            pp(inputs["ln1_g"], CT), pp(inputs["ln1_b"], CT),
            pp(inputs["ln2_g"], CT), pp(inputs["ln2_b"], CT),
            pp(inputs["proj_b"], CT), pp(inputs["fc2_b"], CT),
            pp(inputs["fc1_b"], HCT), pp(inputs["dw_b"], HCT),
            dwtap,
        ],
        axis=1,
    )
    # fp8 diagonal conv weights for the DR tap-pair matmuls:
    # dwdiag8[k, hc, pair, plane, m] = w[hc*128+k, ky, kx] * W8SCALE if m == k
    w9 = np.asarray(inputs["dw_w"], np.float32).reshape(HCT, P, 3, 3)
    dwdiag8 = np.zeros((P, HCT, 5, 2, P), np.float32)
    kk = np.arange(P)
    for hc in range(HCT):
        for n, (_q0, _x0, tA, tB) in enumerate(CV_PAIRS):
            for q, tap in enumerate((tA, tB)):
                if tap is None:
                    continue
                ky, kx = tap
                dwdiag8[kk, hc, n, q, kk] = w9[hc, :, ky, kx] * W8SCALE
    def to8(wT, nsub):
        # [nsub*P, cols] -> [P, nsub, cols] fp8, scaled up by W8SCALE
        cols = wT.shape[1]
        return np.ascontiguousarray(
            (wT * W8SCALE).reshape(nsub, P, cols).transpose(1, 0, 2)
        ).astype(ml_dtypes.float8_e4m3)

    w = {
        "qkv_w8": to8(qkv_wT, CT),
        "proj_w8": np.ascontiguousarray(
            np.asarray(inputs["proj_w"], np.float32).T.reshape(2, 2, P, C)
            .transpose(2, 0, 1, 3) * W8SCALE
        ).astype(ml_dtypes.float8_e4m3),
        "biases": np.ascontiguousarray(biases),
        "dwdiag8": np.ascontiguousarray(dwdiag8).astype(ml_dtypes.float8_e4m3),
        "zeros8": np.zeros((P, 3, CV_S), ml_dtypes.float8_e4m3),
        "fc1_w8": to8(np.asarray(inputs["fc1_w"], np.float32).T, CT),
        "fc2_w8": to8(np.asarray(inputs["fc2_w"], np.float32).T, HCT),
    }
    return w


def kernel(**inputs):
    x = np.asarray(inputs["x"], np.float32)  # [8, 1024, 512]
    assert x.shape == (N_CORES, NTOK, C), x.shape
    w = prep_weights(inputs)
    if "nc" not in _CACHE:
        _CACHE["nc"] = build_program()
    nc = _CACHE["nc"]
    in_maps = [prep_core_inputs(x[i], w) for i in range(N_CORES)]
    res = bass_utils.run_bass_kernel_spmd(nc, in_maps, core_ids=list(range(N_CORES)))
    global LAST_RESULT
    LAST_RESULT = res
    out = np.stack(
        [
            np.asarray(res.results[i]["out"])
            .transpose(1, 0, 2)
            .reshape(NTOK, C)
            for i in range(N_CORES)
        ],
        axis=0,
    )
    return out.astype(np.float32)



# revision 97
# speedup vs baseline: 1.0184x; 1.0125x over previous
"""Trainium2 Bass kernel for a SegFormer-style transformer block.

Reference computation (per batch element b):
    x  = x + attention(LN1(x))          # 8 heads, d=64, no qkv bias
    x  = x + mixffn(LN2(x))             # fc1 -> dwconv3x3 -> gelu -> fc2

Sharding: pure data-parallel over batch B=8 across the 8 NeuronCores
(one batch element per core, weights replicated, no collectives).

Per-core layout strategy (v3):
  - LayerNorm stats run token-major; the transpose to feature-major goes
    through the DMA XBAR (dma_start_transpose, 16x128 tiles, two token
    tiles per transfer) instead of the PE.
  - Attention processes heads in PAIRS: head 2t lives on partitions 0:64,
    head 2t+1 on 64:128.  Score matmuls (K=64) for the two heads run
    CONCURRENTLY in the PE array via row tile_position (0,0)/(64,0);
    A@V matmuls (M=64) run concurrently via col tile_position (0,0)/(0,64).
    Softmax denominators come from an all-ones [128,64] stationary matmul
    (output = denominator broadcast across 64 partitions, same PSUM bank
    as A@V's pair so the approximate reciprocal runs at base partition 0).
  - The attention is software-pipelined: the exp evacuations pace the
    phase, so pair t's scores interleave with pair t-1's A@V/denominator
    matmuls (pair 0 interleaves with the V projection); E tiles are
    double-buffered across pairs.
  - proj contracts the full 128-dim head pair in one accumulation chain.
  - The depthwise 3x3 conv runs on the PE as 9 diagonal-matrix matmuls
    accumulating in PSUM; diag tiles are built on the DVE in bf16 (4x).
  - PSUM budget (8 banks): pbig 2x[P,1024] + pav 4x[P,512]; matmul
    chains pair both query chunks into one pbig tile so every ACT/DVE
    evacuation is a single 1024-wide instruction (ACT has ~352 cycles of
    fixed overhead per instruction).
  - All matmuls in bf16 (fp32 PSUM accumulation).

Self-contained: hardcodes all shapes; takes full inputs, returns full
output.
"""

import numpy as np
import ml_dtypes

import concourse.bass as bass
import concourse.tile as tile
from concourse import bacc, mybir
from concourse import bass_utils
from concourse.bass import ts, ds
from concourse.masks import make_identity

P = 128
NTOK = 1024
C = 512
HID = 2048
NH = 8
NP = NH // 2        # head pairs
D = 64
HH = 32
WW = 32
NT = NTOK // P      # 8 token tiles
CT = C // P         # 4 feature tiles
HCT = HID // P      # 16 hidden tiles
EPS = 1e-5
N_CORES = 8

f32 = mybir.dt.float32
bf16 = mybir.dt.bfloat16
fp8 = mybir.dt.float8e4
u8 = mybir.dt.uint8
# Schraudolph fast-exp constants: trunc(x*8/ln2 + 56.16) as uint8 bits IS
# fp8e4m3(~e^x); beta tuned for min max log-ratio error under truncation
SCH_ALPHA = 8.0 / 0.6931471805599453
SCH_BETA = 56.16
AF = mybir.ActivationFunctionType
OP = mybir.AluOpType
DR = mybir.MatmulPerfMode.DoubleRow
W8SCALE = 16.0  # fp8 weights are scaled up x16 (host) and back /16 at evac

# ---- depthwise conv v2: fp8 DoubleRow tap-pair matmuls ----
# ht tile layout (per hidden chunk): [P, 3 planes, CV_S] fp8.
#   plane1 = original zero-padded 34x34 grid at byte CV_G (written by the
#            fc1 evacuation, interior rows/cols 1..32 only; pads stay zero
#            from the one-time whole-tile memset).
#   plane0 = plane1 shifted by +2 elements (DMA copy), plane2 = shifted -34.
# A DR matmul reads two planes at the same in-plane offset x0 (middle-dim
# stride CV_S must be %16==0), so one matmul applies TWO taps:
#   via plane1 (orig):    tap delta = x0 - CV_G
#   via plane0 (shift+2): tap delta = x0 - CV_G + 2
#   via plane2 (shift-34): tap delta = x0 - CV_G - 34
# tap (ky,kx) -> delta = (ky-1)*34 + (kx-1) on the padded grid.
CV_S = 1232            # plane stride (fp8 bytes), 77*16
CV_G = 64              # grid origin within a plane (apron before it)
CV_NPAD = 34 * 34      # padded grid size (1156)
# (first_ap_plane, x0, tap_on_q0, tap_on_q1); taps as (ky,kx), None = zeros
CV_PAIRS = [
    (0, 29, (0, 2), (0, 0)),   # deltas -33, -35
    (1, 64, (1, 1), (0, 1)),   # planes (1,2): deltas 0, -34
    (0, 63, (1, 2), (1, 0)),   # deltas +1, -1
    (0, 97, (2, 2), (2, 0)),   # deltas +35, +33
    (0, 98, None, (2, 1)),     # deltas (+36 x zeros), +34
]


def _emit(tc, d, out_ap, gelu_mode="hw", dbg=None):
    def dump(key, ap):
        if dbg is not None and key in dbg:
            tc.nc.sync.dma_start(dbg[key], ap)

    nc = tc.nc

    # ---- whole-kernel pools ----
    pool_const = tc.alloc_tile_pool(name="const", bufs=1)
    pool_x = tc.alloc_tile_pool(name="x", bufs=1)
    pool_x1 = tc.alloc_tile_pool(name="x1", bufs=1)
    pool_stats = tc.alloc_tile_pool(name="stats", bufs=4)
    pool_out = tc.alloc_tile_pool(name="outp", bufs=1)
    # PSUM budget (8 banks): pbig 2x[P,1024] + pav (av,dn) x2 [P,512]
    pool_pbig = tc.alloc_tile_pool(name="pbig", bufs=2, space="PSUM")
    pool_pav = tc.alloc_tile_pool(name="pav", bufs=2, space="PSUM")

    identb = pool_const.tile([P, P], bf16, tag="identb", name="identb")
    make_identity(nc, identb[:])
    # fp8 all-ones stationary for the denominator matmuls
    ones8r = pool_const.tile([P, 2, D], fp8, tag="ones8r", name="ones8r")
    nc.vector.memset(ones8r[:], 1.0)
    zconst = pool_const.tile([P, 1], f32, tag="zconst", name="zconst")
    nc.vector.memset(zconst[:], 0.0)
    nc.const_aps.aps[(f32, 0.0)] = zconst[:]
    epsap = pool_const.tile([P, 1], f32, tag="epsap", name="epsap")
    nc.vector.memset(epsap[:], EPS)

    # all per-partition bias vectors arrive host-packed in one [P, 56] DMA
    # (small, ahead of the x tiles on the sync queue)
    bias_sb = pool_const.tile([P, 6 * CT + 3 * HCT], f32, tag="biases", name="biases")
    nc.sync.dma_start(bias_sb[:], d["biases"])
    g1pp = bias_sb[:, 0:4]
    b1pp = bias_sb[:, 4:8]
    g2pp = bias_sb[:, 8:12]
    b2pp = bias_sb[:, 12:16]
    pbpp = bias_sb[:, 16:20]
    f2bpp = bias_sb[:, 20:24]
    f1bpp = bias_sb[:, 24:40]
    dwbpp = bias_sb[:, 40:56]
    dwtpp = bias_sb[:, 56:72]
    # host-prebuilt fp8 diagonal conv weights: [P, hc, pair, plane, 128].
    # The tile lives in the const pool but its (bulky) DMA is emitted later,
    # once the latency-critical LN1 transfers have cleared the queues.
    dw8 = pool_const.tile([P, HCT, 5, 2, P], fp8, tag="dw8", name="dw8")
    # persistent triple-buffered conv input tiles; zero-filled once via DMA
    # (engine-free) so pads/aprons/tails read as zeros forever (the fc1 evac
    # only writes interiors, the plane copies rewrite full grids).
    pool_htp = tc.alloc_tile_pool(name="htp", bufs=1, side="right")
    ht_bufs = [
        pool_htp.tile([P, 3, CV_S], fp8, tag=f"htb{i}", name=f"htb{i}")
        for i in range(3)
    ]
    # pre-trigger the sqrt table set (LN rstd) while the input DMAs run
    scr1 = pool_const.tile([P, 1], f32, tag="scr1", name="scr1")
    nc.scalar.activation(scr1[0:1, :], epsap[0:1, :], AF.Sqrt)

    # x arrives host-repacked partition-major bf16 [P, NT, C]: two DMAs with
    # 8KB-contiguous partition lines instead of 1024 2KB descriptors.
    xall = pool_x.tile([P, NT, C], bf16, tag="xall", name="xall")
    nc.sync.dma_start(xall[:, 0:1, :], d["xb"][:, 0:1, :])
    nc.sync.dma_start(xall[:, 1:4, :], d["xb"][:, 1:4, :])
    nc.scalar.dma_start(xall[:, 4:8, :], d["xb"][:, 4:8, :])
    x_sb = [xall[:, i, :] for i in range(NT)]
    x1_sb = [pool_x1.tile([P, C], f32, tag=f"x1_{i}", name=f"x1_{i}") for i in range(NT)]
    # fc1/fc2 weights live in the const pool so their (bulky) DMAs can run
    # during the attention stretch when the queues are idle.
    w18 = pool_const.tile([P, CT, HID], fp8, tag="w18", name="w18")
    w28 = pool_const.tile([P, HCT, C], fp8, tag="w28", name="w28")

    def emit_ln(src_tiles, gpp, bpp, dstT, pool_xn, pairs=None, prestats=None):
        """Token-major LN over C; transpose to feature-major via DMA XBAR
        (two token tiles batched per transpose).

        Wave-structured to avoid per-tile ACT<->DVE ping-pong: all tiles'
        normalization scalars (rstd, nb) are computed first, then the
        normalize+transpose+scale pipeline runs per pair.

        dstT is a single [P, CT, NTOK] bf16 tile: chunk c holds features
        c*128+p on partitions, tokens on the free axis.  prestats, if
        given, maps tile index -> [P, CT, 6] per-chunk bn_stats tile
        (computed earlier, e.g. fused into the residual adds)."""
        ips = list(pairs if pairs is not None else range(NT // 2))
        idxs = [2 * ip + i2 for ip in ips for i2 in range(2)]
        n = len(idxs)
        # batched normalization scalars: aggr per tile into one [P,n,2]
        # buffer, then ONE sqrt / reciprocal / negate-multiply for all tiles
        # (the per-tile versions cost ~0.3us of fixed overhead each)
        mv8 = pool_stats.tile([P, NT, 2], f32, tag="mv8", name="mv8", bufs=2)
        for k, i in enumerate(idxs):
            if prestats is None:
                st6 = pool_stats.tile([P, 6], f32, tag="st6", name="st6")
                nc.vector.bn_stats(st6[:], src_tiles[i][:])
                st6_ap = st6[:]
            else:
                st6_ap = prestats[i][:]
            nc.vector.bn_aggr(mv8[:, k, :], st6_ap)
        sd8 = pool_stats.tile([P, NT], f32, tag="sd8", name="sd8", bufs=2)
        nc.scalar.activation(
            sd8[:, 0:n], mv8[:, 0:n, 1], AF.Sqrt, bias=epsap[:, 0:1]
        )
        rstd8 = pool_stats.tile([P, NT], f32, tag="rstd8", name="rstd8", bufs=2)
        nc.vector.reciprocal(rstd8[:, 0:n], sd8[:, 0:n])
        nb8 = pool_stats.tile([P, NT], f32, tag="nb8", name="nb8", bufs=2)
        nc.vector.scalar_tensor_tensor(
            nb8[:, 0:n], mv8[:, 0:n, 0], -1.0, rstd8[:, 0:n], OP.mult, OP.mult
        )
        rstds = {i: rstd8[:, k : k + 1] for k, i in enumerate(idxs)}
        nbs = {i: nb8[:, k : k + 1] for k, i in enumerate(idxs)}
        for ip in ips:
            xn2 = pool_xn.tile([P, 2, C], bf16, tag="xn", name="xn", bufs=2)
            for i2 in range(2):
                i = 2 * ip + i2
                nc.scalar.activation(
                    xn2[:, i2, :], src_tiles[i][:], AF.Identity,
                    bias=nbs[i], scale=rstds[i],
                )
            # transpose on the PE (idle during LN; ~8x faster than the DMA
            # XBAR): 8 [128,128] blocks into one 1-bank bf16 psum tile
            xr = pool_pbig.tile([P, 2, CT, P], bf16, tag="big", name="xr")
            for i2 in range(2):
                for c in range(CT):
                    nc.tensor.transpose(
                        xr[:, i2, c, :], xn2[:, i2, ts(c, P)], identb[:]
                    )
            for c in range(CT):
                nc.vector.tensor_scalar(
                    dstT[:, c, ts(ip, 2 * P)].rearrange("p (a b) -> p a b", a=2),
                    xr[:, :, c, :],
                    gpp[:, c : c + 1],
                    bpp[:, c : c + 1],
                    OP.mult,
                    OP.add,
                )

    # ================= LN1 + QKV (fp8 DoubleRow) =================
    pool_a = tc.alloc_tile_pool(name="poolA", bufs=1)
    wq8 = pool_a.tile([P, CT, 3 * C], fp8, tag="wq8", name="wq8")
    # scalar queue, behind the second x half
    nc.scalar.dma_start(wq8[:], d["qkv_w8"])
    xlnT = pool_a.tile([P, CT, NTOK], fp8, tag="xlnT", name="xlnT")

    pool_b = tc.alloc_tile_pool(name="poolB", bufs=1, side="right")
    Q_sb = [pool_b.tile([P, NTOK], bf16, tag=f"Q{t}", name=f"Q{t}") for t in range(NP)]
    K_sb = [pool_b.tile([P, NTOK], bf16, tag=f"K{t}", name=f"K{t}") for t in range(NP)]
    # V in fp8, one tile with key-tile-major layout
    V2 = pool_b.tile([P, NT, NH, D], fp8, tag="V2", name="V2")
    # E tiles (fp8, merged per parity so j-pairs sit at a %16 plane stride
    # for DR): head A of each pair gets exact ACT exp, head B gets a
    # Schraudolph fast-exp on the DVE (uint8 bitcast -> fp8e4m3)
    EAb = [pool_b.tile([P, NT, NTOK], fp8, tag=f"EAb{p}", name=f"EAb{p}")
           for p in range(2)]
    EBb = [pool_b.tile([P, NT, NTOK], fp8, tag=f"EBb{p}", name=f"EBb{p}")
           for p in range(2)]

    def emit_qk(nqc):
        # Q/K head-pair chunks (qkv_wT host-reordered [Qpairs|Kpairs|V]):
        # psum rows 0-63 = head 2t, 64-127 = head 2t+1.
        for t in range(NP):
            for dst, base, tg in ((Q_sb, 0, "av"), (K_sb, C, "dn")):
                ps = pool_pav.tile([P, 512], f32, tag=tg, name=tg)
                for cp in range(CT // 2):
                    nc.tensor.matmul(
                        ps[:],
                        wq8[:, 2 * cp : 2 * cp + 2, base + t * P : base + (t + 1) * P],
                        xlnT[:, 2 * cp : 2 * cp + 2, ts(nqc, 512)],
                        start=(cp == 0),
                        stop=(cp == CT // 2 - 1),
                        perf_mode=DR,
                    )
                nc.vector.tensor_scalar(
                    dst[t][:, ts(nqc, 512)], ps[:], 1.0 / W8SCALE, 0.0,
                    OP.mult, OP.add,
                )

    def warm_pe(srcs):
        # staggered dummy matmuls that each wait on a freshly-produced tile
        # chunk: keeps the PE's HAM activity window busy across PE-idle
        # stretches so the real matmuls that follow start at full clock.
        for n, src in enumerate(srcs):
            ps = pool_pav.tile([P, 512], f32, tag=("av", "dn")[n % 2], name="wm")
            nc.tensor.matmul(ps[0:D, 0:64], src, src)

    emit_ln(x_sb, g1pp, b1pp, xlnT, pool_a)
    warm_pe([xlnT[:, c, 0:64] for c in (0, 2)])
    emit_qk(0)
    emit_qk(1)
    dump("xlnT", xlnT[:])

    def emit_V(j):
        # V in token-major: [tok, (h, dv)]
        ps = pool_pav.tile([P, 512], f32, tag="av", name="av")
        for cp in range(CT // 2):
            nc.tensor.matmul(
                ps[:],
                xlnT[:, 2 * cp : 2 * cp + 2, ts(j, P)],
                wq8[:, 2 * cp : 2 * cp + 2, 2 * C : 3 * C],
                start=(cp == 0),
                stop=(cp == CT // 2 - 1),
                perf_mode=DR,
            )
        nc.vector.tensor_scalar(
            V2[:, j, :, :], ps[:].rearrange("p (h r) -> p h r", h=NH),
            1.0 / W8SCALE, 0.0, OP.mult, OP.add,
        )

    # ================= attention (software-pipelined head pairs) ==========
    pend = {}

    def avdn_j(t, j):
        """A@V + ones-denominator matmuls for pair t, key tile j (both
        query chunks).  Heads share banks: A rows 0:64, B rows 64:128.
        (Non-DR: M=64 outputs column-tile to ~1.4x rate.)"""
        if t not in pend:
            pend[t] = (
                [pool_pav.tile([P, 512], f32, tag="av", name="av") for _ in range(2)],
                [pool_pav.tile([P, 512], f32, tag="dn", name="dn") for _ in range(2)],
            )
        avs, dns = pend[t]
        st, sp = (j == 0), (j == NT - 1)
        par = t % 2
        for nqc in range(2):
            ea = EAb[par][:, j, ts(nqc, 512)]
            eb = EBb[par][:, j, ts(nqc, 512)]
            av, dn = avs[nqc], dns[nqc]
            nc.tensor.matmul(av[0:D, :], V2[:, j, 2 * t, :], ea, start=st, stop=sp,
                             skip_group_check=True)
            nc.tensor.matmul(av[D : 2 * D, :], V2[:, j, 2 * t + 1, :], eb,
                             start=st, stop=sp, skip_group_check=True)
            nc.tensor.matmul(dn[0:D, :], ones8r[:, 0, :], ea, start=st, stop=sp,
                             skip_group_check=True)
            nc.tensor.matmul(dn[D : 2 * D, :], ones8r[:, 0, :], eb, start=st,
                             stop=sp, skip_group_check=True)

    def finalize(t):
        """Normalize pair t: OT = av / dn (approx-reciprocal + multiply)."""
        avs, dns = pend.pop(t)
        for nqc in range(2):
            dsx = pool_c.tile([P, 512], f32, tag=f"ds{nqc}", name=f"ds{nqc}")
            nc.vector.reciprocal_approx_fast(out=dsx[:], in_=dns[nqc][:])
            nc.vector.tensor_tensor(
                OT2[:, t, ts(nqc, 512)], avs[nqc][:], dsx[:], OP.mult
            )
            if t == 0 and nqc == 1:
                dump("dsA0", dsx[:])
        if t == 0:
            dump("EA0", EAb[0][:, 0, :])

    def scores_j(t, j):
        pbA = pool_pbig.tile([P, NTOK], f32, tag="big", name="big")
        pbB = pool_pbig.tile([P, NTOK], f32, tag="big", name="big")
        for nq in range(2):
            nc.tensor.matmul(
                pbA[:, ts(nq, 512)], K_sb[t][0:D, ts(j, P)],
                Q_sb[t][0:D, ts(nq, 512)],
            )
            nc.tensor.matmul(
                pbB[:, ts(nq, 512)], K_sb[t][D : 2 * D, ts(j, P)],
                Q_sb[t][D : 2 * D, ts(nq, 512)],
            )
        par = t % 2
        nc.scalar.activation(EAb[par][:, j, :], pbA[:], AF.Exp)
        if t % 2 == 1:
            # Schraudolph fast-exp on the DVE for odd pairs' B heads:
            # trunc(alpha*x + beta) as uint8 IS the fp8e4m3 pattern of ~e^x
            # (scores in [-1.3,1.3], far from the uint8 wrap thresholds).
            nc.vector.tensor_scalar(
                EBb[par][:, j, :].bitcast(u8), pbB[:], SCH_ALPHA, SCH_BETA,
                OP.mult, OP.add,
            )
        else:
            nc.scalar.activation(EBb[par][:, j, :], pbB[:], AF.Exp)

    # pair 0: fill the exp-paced gaps with the V projection, then free
    # the LN1/qkv-weight pool before allocating the attention-output pool.
    for j in range(NT):
        scores_j(0, j)
        emit_V(j)
    # bulk weight/zero loads on the sync queue: it is idle for the whole
    # attention stretch, and none of these are needed until proj/MixFFN.
    # (They must not ride the scalar queue -- dma issue occupies the ACT
    # sequencer, which is saturated with softmax exp here.)
    nc.sync.dma_start(w18[:], d["fc1_w8"])
    nc.sync.dma_start(dw8[:], d["dwdiag8"])
    nc.sync.dma_start(w28[:], d["fc2_w8"])
    for t in ht_bufs:
        nc.sync.dma_start(t[:], d["zeros8"])
    pool_a.release()

    pool_c = tc.alloc_tile_pool(name="poolC", bufs=1)
    OT2 = pool_c.tile([P, NP, NTOK], fp8, tag="OT2", name="OT2")
    # fp8 proj weights with pair planes adjacent: DoubleRow halves the
    # contraction passes (output is full-width M=128, so DR forfeits nothing)
    pw8s = pool_c.tile([P, 2, 2, C], fp8, tag="pw8s", name="pw8s")
    nc.sync.dma_start(pw8s[:], d["proj_w8"])
    PT_sb = [pool_c.tile([P, NTOK], bf16, tag=f"PT{c}", name=f"PT{c}") for c in range(CT)]

    for t in range(1, NP):
        for j in range(NT):
            scores_j(t, j)
            avdn_j(t - 1, j)
        finalize(t - 1)
    for j in range(NT):
        avdn_j(NP - 1, j)
    finalize(NP - 1)

    dump("OT0", OT2[:, 0, :])
    pool_b.release()

    # ===== proj + residual 1 + per-chunk LN2 stats (fused per c-chunk) =====
    # proj accumulates in the pav banks (2x[P,512] per oc) so the pbig ring
    # is free for the PE transposes of PT; all proj chains are emitted before
    # the first transpose so the PE never waits on an evac.
    st6_2 = [pool_stats.tile([P, CT, 6], f32, tag=f"st2_{i}", name=f"st2_{i}")
             for i in range(NT)]
    for oc in range(CT):
        for nqc in range(2):
            pq = pool_pav.tile([P, 512], f32, tag=("av", "dn")[nqc], name="pj")
            for m2 in range(2):
                nc.tensor.matmul(
                    pq[:],
                    pw8s[:, m2, :, ts(oc, P)],
                    OT2[:, 2 * m2 : 2 * m2 + 2, ts(nqc, 512)],
                    start=(m2 == 0),
                    stop=(m2 == 1),
                    perf_mode=DR,
                )
            nc.scalar.activation(
                PT_sb[oc][:, ts(nqc, 512)], pq[:], AF.Identity,
                bias=pbpp[:, oc : oc + 1], scale=1.0 / W8SCALE,
            )
    for oc in range(CT):
        ptp = pool_pbig.tile([P, NT, P], bf16, tag="big", name="ptp")
        for i in range(NT):
            nc.tensor.transpose(ptp[:, i, :], PT_sb[oc][:, ts(i, P)], identb[:])
        for i in range(NT):
            nc.vector.tensor_tensor(
                x1_sb[i][:, ts(oc, P)], ptp[:, i, :], x_sb[i][:, ts(oc, P)], OP.add
            )
            nc.vector.bn_stats(st6_2[i][:, oc, :], x1_sb[i][:, ts(oc, P)])

    dump("PT0", PT_sb[0][:])
    pool_c.release()

    # ================= LN2 =================
    pool_d = tc.alloc_tile_pool(name="poolD", bufs=2)
    x2T = pool_d.tile([P, CT, NTOK], fp8, tag="x2T", name="x2T")
    dump("x1_0", x1_sb[0][:])
    warm_pe([x1_sb[2][:, 0:64], x1_sb[5][:, 0:64]])
    emit_ln(x1_sb, g2pp, b2pp, x2T, pool_d, prestats=st6_2)
    warm_pe([x2T[:, c, 0:64] for c in (0, 2)])
    dump("x2T", x2T[:])
    if gelu_mode == "hw":
        # pre-trigger the gelu table set; overlaps the fc1 matmul chains
        nc.scalar.activation(scr1[0:1, :], epsap[0:1, :], AF.Gelu)

    # ================= fc1 + dwconv + gelu (fused per tile) ====
    # PSUM re-plan for the MixFFN: fc1 gets 2x[P,512] (2 banks), the conv
    # psum needs [P,1156] (3 banks) x2 bufs = 6 banks.
    pool_pav.release()
    pool_pbig.release()
    pool_pf = tc.alloc_tile_pool(name="pf", bufs=1, space="PSUM")
    pool_pconv = tc.alloc_tile_pool(name="pconv", bufs=2, space="PSUM")

    pool_e = tc.alloc_tile_pool(name="poolE", bufs=1, side="right")
    Gall = pool_e.tile([P, HCT, NTOK], fp8, tag="Gall", name="Gall")

    def emit_fc1(hc):
        # both query halves into one 2-bank psum tile -> a single wide evac
        ht = ht_bufs[hc % 3]
        ps = pool_pf.tile([P, NTOK], f32, tag="f1", name="f1")
        for nqc in range(2):
            for cp in range(CT // 2):
                nc.tensor.matmul(
                    ps[:, ts(nqc, 512)],
                    w18[:, 2 * cp : 2 * cp + 2, ts(hc, P)],
                    x2T[:, 2 * cp : 2 * cp + 2, ts(nqc, 512)],
                    start=(cp == 0),
                    stop=(cp == CT // 2 - 1),
                    perf_mode=DR,
                )
        # evac on the DVE (ACT is gelu-bound): interior rows 1..32 cols 1..32
        # of plane1's padded grid, one 1024-wide op.
        a0 = CV_G + 34 + 1
        dst = ht[:, 1, a0 : a0 + 32 * 34].rearrange("p (y x) -> p y x", x=34)[
            :, :, 0:32
        ]
        nc.vector.tensor_scalar(
            dst,
            ps[:].rearrange("p (y x) -> p y x", x=WW),
            1.0 / W8SCALE,
            f1bpp[:, hc : hc + 1],
            OP.mult,
            OP.add,
        )
        # shifted copies for the DR tap pairs (near-free on idle DMA queues)
        eng, eng2 = (nc.sync, nc.scalar) if hc % 2 == 0 else (nc.scalar, nc.sync)
        eng.dma_start(
            ht[:, 0, CV_G : CV_G + CV_NPAD], ht[:, 1, CV_G + 2 : CV_G + 2 + CV_NPAD]
        )
        eng2.dma_start(
            ht[:, 2, CV_G : CV_G + CV_NPAD], ht[:, 1, CV_G - 34 : CV_G - 34 + CV_NPAD]
        )

    emit_fc1(0)
    for hc in range(HCT):
        # software pipeline: next chunk's fc1 goes on the PE queue BEFORE this
        # chunk's conv so the PE never waits on the evac+copy chain.
        if hc + 1 < HCT:
            emit_fc1(hc + 1)
        ht = ht_bufs[hc % 3]
        # 5 DR tap-pair matmuls apply all 9 taps (psum covers the full padded
        # grid; garbage lands only in pad positions, never read by the evac).
        # Each is split into 512/512/132 chunks: a matmul output cannot cross
        # a PSUM bank boundary.
        pdc = pool_pconv.tile([P, CV_NPAD], f32, tag="pc", name="pc")
        for n, (q0, x0, _tA, _tB) in enumerate(CV_PAIRS):
            for c0, cn in ((0, 512), (512, 512), (1024, CV_NPAD - 1024)):
                mov = ht[:, q0 : q0 + 2, 0:cn].copy()
                mov.offset = mov.offset + x0 + c0
                nc.tensor.matmul(
                    pdc[:, c0 : c0 + cn],
                    dw8[:, hc, n, :, :],
                    mov,
                    start=(n == 0),
                    stop=(n == len(CV_PAIRS) - 1),
                    perf_mode=DR,
                )
        if hc == 0:
            dump("HT0", ht[:])
        pin = pdc[:, 35 : 35 + 32 * 34].rearrange("p (y x) -> p y x", x=34)[
            :, :, 0:32
        ]
        if gelu_mode == "hw":
            nc.scalar.activation(
                Gall[:, hc, :], pin, AF.Gelu, bias=dwbpp[:, hc : hc + 1],
                scale=1.0 / W8SCALE,
            )
            if hc == 0:
                dump("G0", Gall[:, 0, :])
        else:
            # sim-only fallback: gelu(x) ~= x * sigmoid(1.702 x)
            hb = pool_e.tile([P, NTOK], f32, tag="hb", name="hb", bufs=2)
            nc.scalar.activation(
                hb[:], pin, AF.Identity, bias=dwbpp[:, hc : hc + 1],
                scale=1.0 / W8SCALE,
            )
            sg = pool_e.tile([P, NTOK], f32, tag="sg", name="sg", bufs=2)
            nc.scalar.activation(sg[:], hb[:], AF.Sigmoid, scale=1.702)
            nc.vector.tensor_mul(Gall[:, hc, :], hb[:], sg[:])

    pool_pconv.release()
    pool_pf2 = tc.alloc_tile_pool(name="pf2", bufs=2, space="PSUM")

    # ================= fc2 + residual 2 + output (fused per oc) ===========
    pool_d.release()
    pool_f = tc.alloc_tile_pool(name="poolF", bufs=1)
    FT_sb = [pool_f.tile([P, NTOK], bf16, tag=f"FT{c}", name=f"FT{c}") for c in range(CT)]
    # output partition-major [P, NT, C]: the host transposes back; stores go
    # out in two big-descriptor DMAs instead of 1024 2KB lines
    oall = pool_out.tile([P, NT, C], f32, tag="oall", name="oall")
    ot_sb = [oall[:, i, :] for i in range(NT)]
    for oc in range(CT):
        pq = pool_pf2.tile([P, NTOK], f32, tag="f2", name="f2")
        for nqc in range(2):
            for hp in range(HCT // 2):
                nc.tensor.matmul(
                    pq[:, ts(nqc, 512)],
                    w28[:, 2 * hp : 2 * hp + 2, ts(oc, P)],
                    Gall[:, 2 * hp : 2 * hp + 2, ts(nqc, 512)],
                    start=(hp == 0),
                    stop=(hp == HCT // 2 - 1),
                    perf_mode=DR,
                )
        # output path per query half: the nq0 half's evac/transpose/adds run
        # while the nq1 matmuls stream (subtile deps on the psum halves), and
        # the first out-store launches before the last adds finish
        for nqc in range(2):
            nc.scalar.activation(
                FT_sb[oc][:, ts(nqc, 512)], pq[:, ts(nqc, 512)], AF.Identity,
                bias=f2bpp[:, oc : oc + 1], scale=1.0 / W8SCALE,
            )
            ftp = pool_pf2.tile([P, 4, P], bf16, tag="f2t", name="ftp")
            for k in range(4):
                nc.tensor.transpose(
                    ftp[:, k, :], FT_sb[oc][:, ts(4 * nqc + k, P)], identb[:]
                )
            for k in range(4):
                i = 4 * nqc + k
                nc.vector.tensor_tensor(
                    ot_sb[i][:, ts(oc, P)], ftp[:, k, :], x1_sb[i][:, ts(oc, P)],
                    OP.add,
                )
            if oc == CT - 1:
                nc.sync.dma_start(
                    out_ap[:, 4 * nqc : 4 * nqc + 4, :],
                    oall[:, 4 * nqc : 4 * nqc + 4, :],
                )

    dump("FT0", FT_sb[0][:])
    pool_e.release()
    pool_f.release()
    for p in (pool_pf2, pool_pf, pool_htp, pool_out, pool_stats,
              pool_x1, pool_x, pool_const):
        p.release()


_SHAPES = {
    "biases": (P, 6 * CT + 3 * HCT),
}
_BF16_SHAPES = {
    "xb": (P, NT, C),
}
_FP8_SHAPES = {
    "qkv_w8": (P, CT, 3 * C),
    "fc1_w8": (P, CT, HID),
    "fc2_w8": (P, HCT, C),
    "dwdiag8": (P, HCT, 5, 2, P),
    "zeros8": (P, 3, CV_S),
    "proj_w8": (P, 2, 2, C),
}


DBG_SPECS = {
    "xlnT": ((P, CT, NTOK), "fp8"),
    "EA0": ((P, NTOK), "fp8"),
    "dsA0": ((P, 512), "f32"),
    "OT0": ((P, NTOK), "fp8"),
    "PT0": ((P, NTOK), "bf16"),
    "x1_0": ((P, C), "f32"),
    "x2T": ((P, CT, NTOK), "fp8"),
    "HT0": ((P, 3, CV_S), "fp8"),
    "G0": ((P, NTOK), "fp8"),
    "FT0": ((P, NTOK), "bf16"),
}
_DBG_DT = {"bf16": bf16, "f32": f32, "fp8": fp8}


def build_program(gelu_mode="hw", dbg=False):
    nc = bacc.Bacc(
        "TRN2",
        target_bir_lowering=False,
        debug=False,
        enable_asserts=False,
        num_devices=N_CORES,
    )
    d = {}
    for name, shape in _SHAPES.items():
        d[name] = nc.dram_tensor(name, list(shape), f32, kind="ExternalInput").ap()
    for name, shape in _BF16_SHAPES.items():
        d[name] = nc.dram_tensor(name, list(shape), bf16, kind="ExternalInput").ap()
    for name, shape in _FP8_SHAPES.items():
        d[name] = nc.dram_tensor(name, list(shape), fp8, kind="ExternalInput").ap()
    out_ap = nc.dram_tensor("out", [P, NT, C], f32, kind="ExternalOutput").ap()
    dbg_aps = None
    if dbg:
        dbg_aps = {}
        for k, (shape, dt_) in DBG_SPECS.items():
            dbg_aps[k] = nc.dram_tensor(
                f"dbg_{k}", list(shape), _DBG_DT[dt_],
                kind="ExternalOutput",
            ).ap()
    with tile.TileContext(nc) as tc:
        _emit(tc, d, out_ap, gelu_mode=gelu_mode, dbg=dbg_aps)
    nc.compile()
    return nc


_CACHE = {}
LAST_RESULT = None


def prep_core_inputs(x_b, w):
    """Per-core input map: x_b is this core's [1024, 512] batch slice,
    w the shared host-prepped weight dict."""
    xb = np.ascontiguousarray(
        np.asarray(x_b, np.float32).reshape(NT, P, C).transpose(1, 0, 2)
    ).astype(ml_dtypes.bfloat16)
    m = {"xb": xb}
    m.update(w)
    return m


def prep_weights(inputs):
    qkv_raw = np.asarray(inputs["qkv_w"], np.float32).T  # [C, 3C], head-interleaved
    # reorder columns to [Qpair0..3 | Kpair0..3 | V(head-major)], folding the
    # 1/sqrt(d) score scale into the q columns
    qkv_wT = np.empty((C, 3 * C), np.float32)
    for h in range(NH):
        qcol = qkv_raw[:, h * 3 * D : h * 3 * D + D] * (D ** -0.5)
        kcol = qkv_raw[:, h * 3 * D + D : h * 3 * D + 2 * D]
        vcol = qkv_raw[:, h * 3 * D + 2 * D : h * 3 * D + 3 * D]
        qkv_wT[:, h * D : (h + 1) * D] = qcol
        qkv_wT[:, C + h * D : C + (h + 1) * D] = kcol
        qkv_wT[:, 2 * C + h * D : 2 * C + (h + 1) * D] = vcol
    def pp(v, cols):
        # [cols*P] vector -> [P, cols] per-partition layout
        return np.asarray(v, np.float32).reshape(cols, P).T

    # tap (2,1) per-partition weights (x16 to match the scaled conv psum)
    w9b = np.asarray(inputs["dw_w"], np.float32).reshape(HCT, P, 3, 3)
    dwtap = np.ascontiguousarray(w9b[:, :, 2, 1].T * W8SCALE)  # [P, HCT]
    biases = np.concatenate(
        [
            pp(inputs["ln1_g"], CT), pp(inputs["ln1_b"], CT),
            pp(inputs["ln2_g"], CT), pp(inputs["ln2_b"], CT),
            pp(inputs["proj_b"], CT), pp(inputs["fc2_b"], CT),
            pp(inputs["fc1_b"], HCT), pp(inputs["dw_b"], HCT),
            dwtap,
        ],
        axis=1,
    )
    # fp8 diagonal conv weights for the DR tap-pair matmuls:
    # dwdiag8[k, hc, pair, plane, m] = w[hc*128+k, ky, kx] * W8SCALE if m == k
    w9 = np.asarray(inputs["dw_w"], np.float32).reshape(HCT, P, 3, 3)
    dwdiag8 = np.zeros((P, HCT, 5, 2, P), np.float32)
    kk = np.arange(P)
    for hc in range(HCT):
        for n, (_q0, _x0, tA, tB) in enumerate(CV_PAIRS):
            for q, tap in enumerate((tA, tB)):
                if tap is None:
                    continue
                ky, kx = tap
                dwdiag8[kk, hc, n, q, kk] = w9[hc, :, ky, kx] * W8SCALE
    def to8(wT, nsub):
        # [nsub*P, cols] -> [P, nsub, cols] fp8, scaled up by W8SCALE
        cols = wT.shape[1]
        return np.ascontiguousarray(
            (wT * W8SCALE).reshape(nsub, P, cols).transpose(1, 0, 2)
        ).astype(ml_dtypes.float8_e4m3)

    w = {
        "qkv_w8": to8(qkv_wT, CT),
        "proj_w8": np.ascontiguousarray(
            np.asarray(inputs["proj_w"], np.float32).T.reshape(2, 2, P, C)
            .transpose(2, 0, 1, 3) * W8SCALE
        ).astype(ml_dtypes.float8_e4m3),
        "biases": np.ascontiguousarray(biases),
        "dwdiag8": np.ascontiguousarray(dwdiag8).astype(ml_dtypes.float8_e4m3),
        "zeros8": np.zeros((P, 3, CV_S), ml_dtypes.float8_e4m3),
        "fc1_w8": to8(np.asarray(inputs["fc1_w"], np.float32).T, CT),
        "fc2_w8": to8(np.asarray(inputs["fc2_w"], np.float32).T, HCT),
    }
    return w


def kernel(**inputs):
    x = np.asarray(inputs["x"], np.float32)  # [8, 1024, 512]
    assert x.shape == (N_CORES, NTOK, C), x.shape
    w = prep_weights(inputs)
    if "nc" not in _CACHE:
        _CACHE["nc"] = build_program()
    nc = _CACHE["nc"]
    in_maps = [prep_core_inputs(x[i], w) for i in range(N_CORES)]
    res = bass_utils.run_bass_kernel_spmd(nc, in_maps, core_ids=list(range(N_CORES)))
    global LAST_RESULT
    LAST_RESULT = res
    out = np.stack(
        [
            np.asarray(res.results[i]["out"])
            .transpose(1, 0, 2)
            .reshape(NTOK, C)
            for i in range(N_CORES)
        ],
        axis=0,
    )
    return out.astype(np.float32)

